# revision 1
# baseline (speedup 1.0000x reference)
"""CoreHybridBlock Trainium2 kernel: builder + host glue (work version).

Per-core program (one batch element per core):
  natural layout = [token(part), feature(free)], transposed = [feature(part), token(free)]

  per chunk of C tokens:
    load x,v natural -> rmsnorm(x) natural -> PE-transpose -> xnT
    uvT   = Wconv^T @ xnT   (conv in  proj, 2*dc=512 rows)
    conv  = depthwise K=3 along free dim via tensor_scalar FMA chain + silu gate
    xssmT/dtT/BmT/CmT from matmuls with xnT
    dt chain: clip/softplus/clip ; decay = exp(dt*A) via ACT Exp(scale=A)
    B/C row-norm via ones-matmul partition reduce + bcast matmul
    inp = dt*x_ssm*Bfull ; hsT = tensor_tensor_scan(decay, inp) ; state chains chunks
    yT  = hs*Cfull + D*x_ssm ; y2T = ssm_out^T @ yT
    mixer natural via operand swap (lhsT = concat acts) ; v_new = beta*v + mixer
    x2 = x + v_new ; rmsnorm(x2) -> transpose -> nT
    FFN: aT/bT = W1^T/W3^T @ nT per kf tile ; h = silu(a)*b
         ffn natural via operand swap (lhsT = h slice, rhs = w2 tile) accumulate
    x_out = x2 + ffn ; DMA v_new, x_out
"""

import ml_dtypes
import numpy as np
import bass_rust
import concourse.bass as bass
import concourse.tile as tile
from concourse import mybir
from concourse.bass_utils import run_bass_kernel_spmd

F32 = mybir.dt.float32
F32R = mybir.dt.float32r
BF16 = mybir.dt.bfloat16
AF = mybir.ActivationFunctionType
OP = mybir.AluOpType

D_MODEL, D_CONV, D_MAMBA = 512, 256, 256
DSTATE, N_HEADS, KCONV, FFN = 64, 4, 3, 2048
EPS = 1e-6


# ---------------------------------------------------------------- wait split
def split_waits(nc, max_w=1):
    """walrus in this container rejects >~1 sync wait per instruction on some
    instruction types (the Tile end-drain carries one wait per live
    semaphore).  Hoist excess waits onto same-engine NoOps placed before the
    offending instruction."""
    cnt = 0
    for f in nc.m.functions:
        for bb in f.blocks:
            new_list = []
            changed = False
            for inst in bb.instructions:
                si = inst.sync_info
                waits = list(si.on_wait) if si is not None and si.on_wait else []
                if len(waits) > max_w:
                    changed = True
                    extra = waits[max_w:]
                    si.on_wait = waits[:max_w]
                    for j in range(0, len(extra), max_w):
                        cnt += 1
                        nop = bass_rust.InstNoOp(
                            name=f"I-waitsplit-{cnt}", ins=[], outs=[]
                        )
                        nop.engine = inst.engine
                        nop.sync_info = bass_rust.SyncInfo(
                            on_wait=extra[j : j + max_w], on_update=[]
                        )
                        new_list.append(nop)
                new_list.append(inst)
            if changed:
                bb.instructions = new_list
    return cnt


# ---------------------------------------------------------------- program
def build_program(L, C, beta, split=True, fast=True):
    """One-core program; SPMD over 8 cores with different x/v slices."""
    NCH = L // C
    NSUB = C // 128  # L-subtiles per chunk (natural layout)
    nc = bass.Bass()

    # ---- dram I/O
    x_d = nc.dram_tensor("x", [L, D_MODEL], F32, kind="ExternalInput")
    v_d = nc.dram_tensor("v", [L, D_MODEL], F32, kind="ExternalInput")
    wconv_d = nc.dram_tensor("w_conv", [D_MODEL, 2 * D_CONV], F32, kind="ExternalInput")
    wxp_d = nc.dram_tensor("w_xproj", [D_MODEL, D_MAMBA], F32, kind="ExternalInput")
    wdt_d = nc.dram_tensor("w_dt", [D_MODEL, D_MAMBA], F32, kind="ExternalInput")
    wb_d = nc.dram_tensor("w_b", [D_MODEL, DSTATE], F32, kind="ExternalInput")
    wc_d = nc.dram_tensor("w_c", [D_MODEL, DSTATE], F32, kind="ExternalInput")
    wssm_d = nc.dram_tensor("w_ssmout", [D_MAMBA, D_MAMBA], F32, kind="ExternalInput")
    wop_d = nc.dram_tensor("w_outproj", [D_MODEL, D_MODEL], F32, kind="ExternalInput")
    w1_d = nc.dram_tensor("w1", [D_MODEL, FFN], BF16, kind="ExternalInput")
    w3_d = nc.dram_tensor("w3", [D_MODEL, FFN], BF16, kind="ExternalInput")
    w2_d = nc.dram_tensor("w2", [FFN, D_MODEL], BF16, kind="ExternalInput")
    avec_d = nc.dram_tensor("a_vec", [D_MAMBA, 1], F32, kind="ExternalInput")
    dtb_d = nc.dram_tensor("dtb_vec", [D_MAMBA, 1], F32, kind="ExternalInput")
    dvec_d = nc.dram_tensor("d_vec", [D_MAMBA, 1], F32, kind="ExternalInput")
    convb_d = nc.dram_tensor("convb_vec", [D_CONV, 1], F32, kind="ExternalInput")
    convw_d = nc.dram_tensor("convw", [D_CONV, KCONV], F32, kind="ExternalInput")
    mask2_d = nc.dram_tensor("mask2", [128, 2], F32, kind="ExternalInput")
    selb_d = nc.dram_tensor("selb", [2, 128], F32, kind="ExternalInput")
    selc_d = nc.dram_tensor("selc", [2, 128], F32, kind="ExternalInput")
    ident_d = nc.dram_tensor("ident", [128, 128], F32, kind="ExternalInput")

    xo_d = nc.dram_tensor("x_out", [L, D_MODEL], F32, kind="ExternalOutput")
    vo_d = nc.dram_tensor("v_out", [L, D_MODEL], F32, kind="ExternalOutput")

    with tile.TileContext(nc) as tc:
        with (
            tc.tile_pool(name="consts", bufs=1) as cp,
            tc.tile_pool(name="state", bufs=1) as sp,
            tc.tile_pool(name="innat", bufs=2 * NSUB) as pin,
            tc.tile_pool(name="norm", bufs=2) as pnorm,
            tc.tile_pool(name="xn", bufs=NSUB + 1) as pxn,
            tc.tile_pool(name="xnT", bufs=8) as pxnT,
            tc.tile_pool(name="convp", bufs=3) as pconv,
            tc.tile_pool(name="ssm", bufs=2) as pssm,
            tc.tile_pool(name="bc", bufs=2) as pbc,
            tc.tile_pool(name="nat2", bufs=2 * NSUB) as pnat2,
            tc.tile_pool(name="ffna", bufs=3) as pffna,
            tc.tile_pool(name="psC", bufs=4, space="PSUM") as psC,
            tc.tile_pool(name="psN", bufs=4, space="PSUM") as psN,
        ):
            MDT = F32R if fast else F32

            def mm(out, lhsT, rhs, start, stop):
                nc.tensor.matmul(out=out, lhsT=lhsT, rhs=rhs, start=start, stop=stop)

            # ---------------- constants / weights resident in SBUF
            def load_const(name, dram_ap, shape, dt=F32):
                t = cp.tile(shape, dt, name=name, tag=name)
                src_ap = dram_ap.bitcast(dt) if dt is F32R else dram_ap
                nc.sync.dma_start(out=t, in_=src_ap)
                return t

            wconv_sb = [
                load_const(f"wconv{k}", wconv_d[k * 128 : (k + 1) * 128, :], [128, 2 * D_CONV], MDT)
                for k in range(4)
            ]
            wxp_sb = [
                load_const(f"wxp{k}", wxp_d[k * 128 : (k + 1) * 128, :], [128, D_MAMBA], MDT)
                for k in range(4)
            ]
            wdt_sb = [
                load_const(f"wdt{k}", wdt_d[k * 128 : (k + 1) * 128, :], [128, D_MAMBA], MDT)
                for k in range(4)
            ]
            wb_sb = [
                load_const(f"wb{k}", wb_d[k * 128 : (k + 1) * 128, :], [128, DSTATE])
                for k in range(4)
            ]
            wc_sb = [
                load_const(f"wc{k}", wc_d[k * 128 : (k + 1) * 128, :], [128, DSTATE])
                for k in range(4)
            ]
            wssm_sb = [
                load_const(f"wssm{k}", wssm_d[k * 128 : (k + 1) * 128, :], [128, D_MAMBA], MDT)
                for k in range(2)
            ]
            wop_sb = [
                load_const(f"wop{k}", wop_d[k * 128 : (k + 1) * 128, :], [128, D_MODEL], MDT)
                for k in range(4)
            ]
            w2_sb = [
                load_const(f"w2_{k}", w2_d[k * 128 : (k + 1) * 128, :], [128, D_MODEL], BF16)
                for k in range(16)
            ]
            w1_sb = [
                load_const(f"w1_{k}", w1_d[k * 128 : (k + 1) * 128, :], [128, FFN], BF16)
                for k in range(4)
            ]
            w3_sb = [
                load_const(f"w3_{k}", w3_d[k * 128 : (k + 1) * 128, :], [128, FFN], BF16)
                for k in range(4)
            ]
            avec = [
                load_const(f"avec{m}", avec_d[m * 128 : (m + 1) * 128, :], [128, 1])
                for m in range(2)
            ]
            dtb = [
                load_const(f"dtb{m}", dtb_d[m * 128 : (m + 1) * 128, :], [128, 1])
                for m in range(2)
            ]
            dvec = [
                load_const(f"dvec{m}", dvec_d[m * 128 : (m + 1) * 128, :], [128, 1])
                for m in range(2)
            ]
            convb = [
                load_const(f"convb{m}", convb_d[m * 128 : (m + 1) * 128, :], [128, 1])
                for m in range(2)
            ]
            convw = [
                load_const(f"convw{m}", convw_d[m * 128 : (m + 1) * 128, :], [128, KCONV])
                for m in range(2)
            ]
            mask2 = load_const("mask2", mask2_d[:, :], [128, 2])
            selb = load_const("selb", selb_d[:, :], [2, 128])
            selc = load_const("selc", selc_d[:, :], [2, 128])
            ident = load_const("ident", ident_d[:, :], [128, 128])

            eps_sb = cp.tile([128, 1], F32, name="eps_sb", tag="eps_sb")
            nc.vector.memset(eps_sb, EPS)
            one_sb = cp.tile([128, 1], F32, name="one_sb", tag="one_sb")
            nc.vector.memset(one_sb, 1.0)

            # ---------------- persistent cross-chunk state
            h_st = [sp.tile([128, 1], F32, name=f"hst{m}", tag=f"hst{m}") for m in range(2)]
            u_halo = [sp.tile([128, 2], F32, name=f"uhalo{m}", tag=f"uhalo{m}") for m in range(2)]
            for m in range(2):
                nc.vector.memset(h_st[m], 0.0)
                nc.vector.memset(u_halo[m], 0.0)

            # ---------------- helpers
            def rmsnorm_apply(src_tiles, dst_tag):
                """fp32 rmsnorm over feature dim (natural layout); weight is
                folded into downstream matmul weights on the host."""
                out_tiles = []
                for i, xt in enumerate(src_tiles):
                    sq = pnorm.tile([128, D_MODEL], F32, name="sq", tag="sq")
                    ssq = pnorm.tile([128, 1], F32, name="ssq", tag="ssq")
                    nc.scalar.activation(out=sq, in_=xt, func=AF.Square, accum_out=ssq)
                    r = pnorm.tile([128, 1], F32, name="rr", tag="rr")
                    nc.scalar.activation(
                        out=r, in_=ssq, func=AF.Sqrt, scale=1.0 / D_MODEL, bias=eps_sb
                    )
                    nc.vector.reciprocal(out=r, in_=r)
                    xn = pxn.tile([128, D_MODEL], F32, name=dst_tag, tag=dst_tag)
                    nc.vector.tensor_scalar(
                        out=xn, in0=xt, scalar1=r, scalar2=None, op0=OP.mult
                    )
                    out_tiles.append(xn)
                return out_tiles

            def transpose_tiles(nat_tiles, dst_tag, dt):
                """[NSUB x [128, D_MODEL]] natural -> 4 x [128, C] transposed."""
                outT = []
                for d in range(4):
                    ps = psC.tile([128, C], F32, name="psC", tag="psC")
                    for i in range(NSUB):
                        nc.tensor.transpose(
                            out=ps[:, i * 128 : (i + 1) * 128],
                            in_=nat_tiles[i][:, d * 128 : (d + 1) * 128],
                            identity=ident,
                        )
                    t = pxnT.tile([128, C], dt, name=dst_tag, tag=dst_tag)
                    nc.vector.tensor_copy(out=t, in_=ps)
                    outT.append(t)
                return outT

            # ---------------- main chunk loop
            for c in range(NCH):
                row0 = c * C

                x_nat, v_nat = [], []
                for i in range(NSUB):
                    xt = pin.tile([128, D_MODEL], F32, name="xnat", tag="xnat")
                    nc.sync.dma_start(
                        out=xt, in_=x_d[row0 + i * 128 : row0 + (i + 1) * 128, :]
                    )
                    x_nat.append(xt)
                    vt = pin.tile([128, D_MODEL], F32, name="vnat", tag="vnat")
                    nc.sync.dma_start(
                        out=vt, in_=v_d[row0 + i * 128 : row0 + (i + 1) * 128, :]
                    )
                    v_nat.append(vt)

                xn_nat = rmsnorm_apply(x_nat, "xn")
                xnT = transpose_tiles(xn_nat, "xnT", MDT)

                # ---- conv input projection: uvT[m] m<4 (u: m 0-1, gate: m 2-3)
                u_ext, g_s = [], []
                for m in range(4):
                    ps = psC.tile([128, C], F32, name="psC", tag="psC")
                    for k in range(4):
                        mm(
                            out=ps,
                            lhsT=wconv_sb[k][:, m * 128 : (m + 1) * 128],
                            rhs=xnT[k],
                            start=(k == 0),
                            stop=(k == 3),
                        )
                    if m < 2:
                        ue = pconv.tile([128, C + 2], F32, name="uext", tag="uext")
                        nc.vector.tensor_copy(out=ue[:, 2 : C + 2], in_=ps)
                        nc.vector.tensor_copy(out=ue[:, 0:2], in_=u_halo[m])
                        nc.vector.tensor_copy(out=u_halo[m], in_=ue[:, C : C + 2])
                        u_ext.append(ue)
                    else:
                        gsig = pconv.tile([128, C], F32, name="gsig", tag="gsig")
                        nc.scalar.activation(out=gsig, in_=ps, func=AF.Sigmoid)
                        gs = pconv.tile([128, C], F32, name="gs", tag="gs")
                        nc.vector.tensor_mul(out=gs, in0=ps, in1=gsig)
                        g_s.append(gs)

                conv_out = []
                for m in range(2):
                    cc = pconv.tile([128, C], F32, name="cc", tag="cc")
                    nc.vector.tensor_scalar(
                        out=cc,
                        in0=u_ext[m][:, 0:C],
                        scalar1=convw[m][:, 0:1],
                        scalar2=convb[m],
                        op0=OP.mult,
                        op1=OP.add,
                    )
                    for kk in (1, 2):
                        nc.vector.scalar_tensor_tensor(
                            out=cc,
                            in0=u_ext[m][:, kk : C + kk],
                            scalar=convw[m][:, kk : kk + 1],
                            in1=cc,
                            op0=OP.mult,
                            op1=OP.add,
                        )
                    co = pconv.tile([128, C], MDT, name="convout", tag="convout")
                    nc.vector.tensor_mul(out=co, in0=cc, in1=g_s[m])
                    conv_out.append(co)

                # ---- x_ssm^T and dt^T and decay^T
                xssmT, dtT, decayT = [], [], []
                for m in range(2):
                    ps = psC.tile([128, C], F32, name="psC", tag="psC")
                    for k in range(4):
                        mm(
                            out=ps,
                            lhsT=wxp_sb[k][:, m * 128 : (m + 1) * 128],
                            rhs=xnT[k],
                            start=(k == 0),
                            stop=(k == 3),
                        )
                    xs = pssm.tile([128, C], F32, name="xssm", tag="xssm")
                    nc.vector.tensor_copy(out=xs, in_=ps)
                    xssmT.append(xs)
                for m in range(2):
                    ps = psC.tile([128, C], F32, name="psC", tag="psC")
                    for k in range(4):
                        mm(
                            out=ps,
                            lhsT=wdt_sb[k][:, m * 128 : (m + 1) * 128],
                            rhs=xnT[k],
                            start=(k == 0),
                            stop=(k == 3),
                        )
                    dt_t = pssm.tile([128, C], F32, name="dtt", tag="dtt")
                    # clip(raw + dt_b, -10, 5)
                    nc.vector.tensor_scalar(
                        out=dt_t, in0=ps, scalar1=dtb[m], scalar2=-10.0,
                        op0=OP.add, op1=OP.max,
                    )
                    nc.vector.tensor_scalar(
                        out=dt_t, in0=dt_t, scalar1=5.0, scalar2=None, op0=OP.min
                    )
                    sp_t = pssm.tile([128, C], F32, name="dtsp", tag="dtsp")
                    nc.scalar.activation(out=sp_t, in_=dt_t, func=AF.Exp)
                    nc.scalar.activation(out=sp_t, in_=sp_t, func=AF.Ln, bias=one_sb)
                    dt_f = pssm.tile([128, C], F32, name="dtf", tag="dtf")
                    nc.vector.tensor_scalar(
                        out=dt_f, in0=sp_t, scalar1=1e-4, scalar2=0.1,
                        op0=OP.max, op1=OP.min,
                    )
                    dtT.append(dt_f)
                    dec = pssm.tile([128, C], F32, name="dec", tag="dec")
                    nc.scalar.activation(out=dec, in_=dt_f, func=AF.Exp, scale=avec[m])
                    decayT.append(dec)

                # ---- B/C projections + row norm
                ps_bc = psC.tile([128, C], F32, name="psC", tag="psC")
                for k in range(4):
                    nc.tensor.matmul(
                        out=ps_bc[0:64, :], lhsT=wb_sb[k], rhs=xnT[k].bitcast(F32),
                        start=(k == 0), stop=(k == 3),
                    )
                for k in range(4):
                    nc.tensor.matmul(
                        out=ps_bc[64:128, :], lhsT=wc_sb[k], rhs=xnT[k].bitcast(F32),
                        start=(k == 0), stop=(k == 3),
                    )
                sq_bc = pbc.tile([128, C], F32, name="sqbc", tag="sqbc")
                nc.scalar.activation(out=sq_bc, in_=ps_bc, func=AF.Square)
                ps_sums = psC.tile([128, C], F32, name="psC", tag="psC")
                nc.tensor.matmul(
                    out=ps_sums[0:2, :], lhsT=mask2, rhs=sq_bc, start=True, stop=True
                )
                r_bc = pbc.tile([2, C], F32, name="rbc", tag="rbc")
                nc.scalar.activation(out=r_bc, in_=ps_sums[0:2, :], func=AF.Sqrt)
                nc.vector.reciprocal(out=r_bc, in_=r_bc)
                nc.vector.tensor_scalar(
                    out=r_bc, in0=r_bc, scalar1=1.0, scalar2=None, op0=OP.min
                )
                bm_s = pbc.tile([128, C], F32, name="bms", tag="bms")
                nc.vector.tensor_copy(out=bm_s, in_=ps_bc)
                ps_sB = psC.tile([128, C], F32, name="psC", tag="psC")
                nc.tensor.matmul(out=ps_sB, lhsT=selb, rhs=r_bc, start=True, stop=True)
                ps_sC = psC.tile([128, C], F32, name="psC", tag="psC")
                nc.tensor.matmul(out=ps_sC, lhsT=selc, rhs=r_bc, start=True, stop=True)
                b128 = pbc.tile([128, C], F32, name="b128", tag="b128")
                c128 = pbc.tile([128, C], F32, name="c128", tag="c128")
                nc.sync.dma_start(out=b128[0:64, :], in_=bm_s[0:64, :])
                nc.sync.dma_start(out=b128[64:128, :], in_=bm_s[0:64, :])
                nc.sync.dma_start(out=c128[0:64, :], in_=bm_s[64:128, :])
                nc.sync.dma_start(out=c128[64:128, :], in_=bm_s[64:128, :])
                nc.vector.tensor_mul(out=b128, in0=b128, in1=ps_sB)
                nc.vector.tensor_mul(out=c128, in0=c128, in1=ps_sC)

                # ---- scan
                yT = []
                for m in range(2):
                    inp = pssm.tile([128, C], F32, name="inp", tag="inp")
                    nc.vector.tensor_mul(out=inp, in0=dtT[m], in1=xssmT[m])
                    nc.vector.tensor_mul(out=inp, in0=inp, in1=b128)
                    hs = pssm.tile([128, C], F32, name="hs", tag="hs")
                    nc.vector.tensor_tensor_scan(
                        out=hs, data0=decayT[m], data1=inp, initial=h_st[m],
                        op0=OP.mult, op1=OP.add,
                    )
                    nc.vector.tensor_copy(out=h_st[m], in_=hs[:, C - 1 : C])
                    hc = pssm.tile([128, C], F32, name="hc", tag="hc")
                    nc.vector.tensor_mul(out=hc, in0=hs, in1=c128)
                    yt = pssm.tile([128, C], MDT, name="yt", tag="yt")
                    nc.vector.scalar_tensor_tensor(
                        out=yt, in0=xssmT[m], scalar=dvec[m], in1=hc,
                        op0=OP.mult, op1=OP.add,
                    )
                    yT.append(yt)

                # ---- ssm out proj
                y2T = []
                for m in range(2):
                    ps = psC.tile([128, C], F32, name="psC", tag="psC")
                    for k in range(2):
                        mm(
                            out=ps,
                            lhsT=wssm_sb[k][:, m * 128 : (m + 1) * 128],
                            rhs=yT[k],
                            start=(k == 0),
                            stop=(k == 1),
                        )
                    y2 = pssm.tile([128, C], MDT, name="y2", tag="y2")
                    nc.vector.tensor_copy(out=y2, in_=ps)
                    y2T.append(y2)

                # ---- mixer (natural layout via operand swap) + velocity/residual
                mix_lhsT = [conv_out[0], conv_out[1], y2T[0], y2T[1]]
                x2_nat = []
                for li in range(NSUB):
                    ps = psN.tile([128, D_MODEL], F32, name="psN", tag="psN")
                    for k in range(4):
                        mm(
                            out=ps,
                            lhsT=mix_lhsT[k][:, li * 128 : (li + 1) * 128],
                            rhs=wop_sb[k],
                            start=(k == 0),
                            stop=(k == 3),
                        )
                    vn = pnat2.tile([128, D_MODEL], F32, name="vnew", tag="vnew")
                    nc.vector.scalar_tensor_tensor(
                        out=vn, in0=v_nat[li], scalar=beta, in1=ps,
                        op0=OP.mult, op1=OP.add,
                    )
                    nc.sync.dma_start(
                        out=vo_d[row0 + li * 128 : row0 + (li + 1) * 128, :], in_=vn
                    )
                    x2 = pnat2.tile([128, D_MODEL], F32, name="x2", tag="x2")
                    nc.vector.tensor_add(out=x2, in0=x_nat[li], in1=vn)
                    x2_nat.append(x2)

                # ---- FFN
                n_nat = rmsnorm_apply(x2_nat, "n2")
                nT = transpose_tiles(n_nat, "nT", BF16)

                ps_ffn = [psN.tile([128, D_MODEL], F32, name="psN", tag="psN") for _ in range(NSUB)]
                for kf in range(16):
                    ps_a = psC.tile([128, C], F32, name="psC", tag="psC")
                    for k in range(4):
                        mm(
                            out=ps_a,
                            lhsT=w1_sb[k][:, kf * 128 : (kf + 1) * 128], rhs=nT[k],
                            start=(k == 0), stop=(k == 3),
                        )
                    ps_b = psC.tile([128, C], F32, name="psC", tag="psC")
                    for k in range(4):
                        mm(
                            out=ps_b,
                            lhsT=w3_sb[k][:, kf * 128 : (kf + 1) * 128], rhs=nT[k],
                            start=(k == 0), stop=(k == 3),
                        )
                    h_a = pffna.tile([128, C], F32, name="ha", tag="ha")
                    nc.scalar.activation(out=h_a, in_=ps_a, func=AF.Sigmoid)
                    h_t = pffna.tile([128, C], F32, name="ht", tag="ht")
                    nc.vector.tensor_mul(out=h_t, in0=ps_a, in1=h_a)
                    h_sb = pffna.tile([128, C], BF16, name="hsb", tag="hsb")
                    nc.vector.tensor_mul(out=h_sb, in0=ps_b, in1=h_t)
                    for li in range(NSUB):
                        mm(
                            out=ps_ffn[li],
                            lhsT=h_sb[:, li * 128 : (li + 1) * 128],
                            rhs=w2_sb[kf],
                            start=(kf == 0),
                            stop=(kf == 15),
                        )

                for li in range(NSUB):
                    xf = pnat2.tile([128, D_MODEL], F32, name="xfin", tag="xfin")
                    nc.vector.tensor_add(out=xf, in0=x2_nat[li], in1=ps_ffn[li])
                    nc.sync.dma_start(
                        out=xo_d[row0 + li * 128 : row0 + (li + 1) * 128, :], in_=xf
                    )

    if split:
        split_waits(nc)
    return nc


# ---------------------------------------------------------------- host glue
def prep_weights(inputs):
    """Host-side preprocessing: fold norm weights into matmul weights,
    precompute A = -exp(A_log), beta, and small constant matrices."""
    f = lambda a: np.asarray(a, dtype=np.float32)
    pre_w = f(inputs["pre_norm_w"])[:, None]
    ffn_w = f(inputs["ffn_norm_w"])[:, None]
    A = -np.exp(f(inputs["A_log"]).reshape(-1))
    mask2 = np.zeros((128, 2), np.float32)
    mask2[0:64, 0] = 1.0
    mask2[64:128, 1] = 1.0
    selb = np.zeros((2, 128), np.float32)
    selb[0, :] = 1.0
    selc = np.zeros((2, 128), np.float32)
    selc[1, :] = 1.0
    beta = float(1.0 / (1.0 + np.exp(-f(inputs["log_beta"]))))
    w = {
        "w_conv": np.ascontiguousarray(pre_w * f(inputs["conv_in_w"])),
        "w_xproj": np.ascontiguousarray(pre_w * f(inputs["x_proj_w"])),
        "w_dt": np.ascontiguousarray(pre_w * f(inputs["dt_w"])),
        "w_b": np.ascontiguousarray(pre_w * f(inputs["B_w"])),
        "w_c": np.ascontiguousarray(pre_w * f(inputs["C_w"])),
        "w_ssmout": np.ascontiguousarray(f(inputs["ssm_out_w"])),
        "w_outproj": np.ascontiguousarray(f(inputs["out_proj_w"])),
        "w1": np.ascontiguousarray((ffn_w * f(inputs["w1"])).astype(ml_dtypes.bfloat16)),
        "w3": np.ascontiguousarray((ffn_w * f(inputs["w3"])).astype(ml_dtypes.bfloat16)),
        "w2": np.ascontiguousarray(f(inputs["w2"]).astype(ml_dtypes.bfloat16)),
        "a_vec": A[:, None].copy(),
        "dtb_vec": f(inputs["dt_b"])[:, None].copy(),
        "d_vec": f(inputs["D"])[:, None].copy(),
        "convb_vec": f(inputs["conv_dw_b"])[:, None].copy(),
        "convw": np.ascontiguousarray(f(inputs["conv_dw_w"])),
        "mask2": mask2,
        "selb": selb,
        "selc": selc,
        "ident": np.eye(128, dtype=np.float32),
    }
    return w, beta


def run(inputs, L=4096, C=256, trace=False):
    w, beta = prep_weights(inputs)
    nc = build_program(L, C, beta)
    x = np.asarray(inputs["x"], np.float32)
    v = np.asarray(inputs["velocity"], np.float32)
    n_cores = x.shape[0]
    in_maps = []
    for b in range(n_cores):
        m = dict(w)
        m["x"] = np.ascontiguousarray(x[b])
        m["v"] = np.ascontiguousarray(v[b])
        in_maps.append(m)
    res = run_bass_kernel_spmd(nc, in_maps, core_ids=list(range(n_cores)), trace=trace)
    x_out = np.stack([res.results[b]["x_out"] for b in range(n_cores)])
    v_out = np.stack([res.results[b]["v_out"] for b in range(n_cores)])
    return (x_out, v_out), res



CHUNK = 256

_PROG_CACHE = {}


def kernel(**inputs):
    """Full-input entry point: shard batch over the 8 NeuronCores (one batch
    element per core; the scan state is per-(batch,channel) so this is
    embarrassingly parallel), run the Bass program SPMD, regather."""
    w, beta = prep_weights(inputs)
    x = np.asarray(inputs["x"], np.float32)
    v = np.asarray(inputs["velocity"], np.float32)
    n_cores, L, _ = x.shape
    key = (L, CHUNK, beta)
    if key not in _PROG_CACHE:
        _PROG_CACHE[key] = build_program(L, CHUNK, beta)
    nc = _PROG_CACHE[key]
    in_maps = []
    for b in range(n_cores):
        m = dict(w)
        m["x"] = np.ascontiguousarray(x[b])
        m["v"] = np.ascontiguousarray(v[b])
        in_maps.append(m)
    res = run_bass_kernel_spmd(nc, in_maps, core_ids=list(range(n_cores)))
    x_out = np.stack([res.results[b]["x_out"] for b in range(n_cores)])
    v_out = np.stack([res.results[b]["v_out"] for b in range(n_cores)])
    return (x_out, v_out)



# revision 12
# speedup vs baseline: 1.3123x; 1.3123x over previous
"""CoreHybridBlock Trainium2 kernel: builder + host glue (v2).

Per-core program (one batch element per core), chunked over tokens (C=512):
  natural layout = [token(part), feature(free)], transposed = [feature(part), token(free)]

  Pipeline is skewed one chunk for the FFN so the scalar engine needs only
  two activation-table switches per iteration (nlexp set <-> silu set):

  iteration i:
    nlexp block: rmsnorm1(i) rsqrt via exp(-ln/2); rmsnorm2(i-1);
                 dt softplus via ln(1+exp); decay exp; B/C rownorm via
                 exp(-relu(ln)/2)  [clip(norm,1) done in log space]
    silu block:  conv gate silu(i); ffn silu(i-1); rmsnorm1-square(i+1)
    PE: xnT(i) transposes, projections(i), nT(i-1) transposes,
        ffn(i-1) w1/w3 + w2(pass li01), ssm_out(i), mixer(i), w2(pass li23)
    DVE: copies/casts, conv FMA chain, scan, gate muls, residual stts
    GpSimd: scan input/output muls, x2 residual add (SBUF-only fp32)

  All matmul operands bf16 (fp32 accumulation in PSUM); residual stream,
  scan, and scalar chains stay fp32.
"""

import ml_dtypes
import numpy as np
import bass_rust
import concourse.bass as bass
import concourse.tile as tile
from concourse import mybir
from concourse.bass_utils import run_bass_kernel_spmd

F32 = mybir.dt.float32
BF16 = mybir.dt.bfloat16
AF = mybir.ActivationFunctionType
OP = mybir.AluOpType

D_MODEL, D_CONV, D_MAMBA = 512, 256, 256
DSTATE, N_HEADS, KCONV, FFN = 64, 4, 3, 2048
EPS = 1e-6
NKF = FFN // 128  # 16


# ---------------------------------------------------------------- wait split
def split_waits(nc, max_w=1):
    """walrus in this container rejects >~1 sync wait per instruction on some
    instruction types (the Tile end-drain carries one wait per live
    semaphore).  Hoist excess waits onto same-engine NoOps placed before the
    offending instruction."""
    cnt = 0
    for f in nc.m.functions:
        for bb in f.blocks:
            new_list = []
            changed = False
            for inst in bb.instructions:
                si = inst.sync_info
                waits = list(si.on_wait) if si is not None and si.on_wait else []
                if len(waits) > max_w:
                    changed = True
                    extra = waits[max_w:]
                    si.on_wait = waits[:max_w]
                    for j in range(0, len(extra), max_w):
                        cnt += 1
                        nop = bass_rust.InstNoOp(
                            name=f"I-waitsplit-{cnt}", ins=[], outs=[]
                        )
                        nop.engine = inst.engine
                        nop.sync_info = bass_rust.SyncInfo(
                            on_wait=extra[j : j + max_w], on_update=[]
                        )
                        new_list.append(nop)
                new_list.append(inst)
            if changed:
                bb.instructions = new_list
    return cnt


# ---------------------------------------------------------------- program
def build_program(L, C, beta, split=True):
    """One-core program; SPMD over 8 cores with different x/v slices."""
    NCH = L // C
    NSUB = C // 128  # token-subtiles per chunk (natural layout)
    nc = bass.Bass()

    # ---- dram I/O
    x_d = nc.dram_tensor("x", [L, D_MODEL], F32, kind="ExternalInput")
    v_d = nc.dram_tensor("v", [L, D_MODEL], F32, kind="ExternalInput")
    wconv_d = nc.dram_tensor("w_conv", [D_MODEL, 2 * D_CONV], BF16, kind="ExternalInput")
    wxp_d = nc.dram_tensor("w_xproj", [D_MODEL, D_MAMBA], BF16, kind="ExternalInput")
    wdt_d = nc.dram_tensor("w_dt", [D_MODEL, D_MAMBA], BF16, kind="ExternalInput")
    wbc_d = nc.dram_tensor("w_bc", [D_MODEL, 2 * DSTATE], BF16, kind="ExternalInput")
    wssm_d = nc.dram_tensor("w_ssmout", [D_MAMBA, D_MAMBA], BF16, kind="ExternalInput")
    wop_d = nc.dram_tensor("w_outproj", [D_MODEL, D_MODEL], BF16, kind="ExternalInput")
    w1_d = nc.dram_tensor("w1", [D_MODEL, FFN], BF16, kind="ExternalInput")
    w3_d = nc.dram_tensor("w3", [D_MODEL, FFN], BF16, kind="ExternalInput")
    w2_d = nc.dram_tensor("w2", [FFN, D_MODEL], BF16, kind="ExternalInput")
    avec_d = nc.dram_tensor("a_vec", [D_MAMBA, 1], F32, kind="ExternalInput")
    dtb_d = nc.dram_tensor("dtb_vec", [D_MAMBA, 1], F32, kind="ExternalInput")
    dvec_d = nc.dram_tensor("d_vec", [D_MAMBA, 1], F32, kind="ExternalInput")
    convb_d = nc.dram_tensor("convb_vec", [D_CONV, 1], F32, kind="ExternalInput")
    convw_d = nc.dram_tensor("convw", [D_CONV, KCONV], F32, kind="ExternalInput")
    mask2_d = nc.dram_tensor("mask2", [128, 2], F32, kind="ExternalInput")
    sel2_d = nc.dram_tensor("sel2", [2, 128], F32, kind="ExternalInput")
    ident_d = nc.dram_tensor("ident", [128, 128], BF16, kind="ExternalInput")

    xo_d = nc.dram_tensor("x_out", [L, D_MODEL], F32, kind="ExternalOutput")
    vo_d = nc.dram_tensor("v_out", [L, D_MODEL], F32, kind="ExternalOutput")

    with tile.TileContext(nc) as tc:
        with (
            tc.tile_pool(name="consts", bufs=1) as cp,
            tc.tile_pool(name="state", bufs=1) as sp,
            tc.tile_pool(name="io", bufs=2) as pio,
            tc.tile_pool(name="act", bufs=2) as pact,
            tc.tile_pool(name="ffn", bufs=2) as pffn,
            tc.tile_pool(name="psT", bufs=2, space="PSUM") as psT,
            tc.tile_pool(name="psP", bufs=4, space="PSUM") as psP,
            tc.tile_pool(name="psF", bufs=2, space="PSUM") as psF,
        ):
            def mm(out, lhsT, rhs, start, stop):
                nc.tensor.matmul(out=out, lhsT=lhsT, rhs=rhs, start=start, stop=stop)

            # ---------------- constants / weights resident in SBUF
            def load_const(name, dram_ap, shape, dt=F32):
                t = cp.tile(shape, dt, name=name, tag=name)
                nc.sync.dma_start(out=t, in_=dram_ap)
                return t

            wconv_sb = [
                load_const(f"wconv{k}", wconv_d[k * 128 : (k + 1) * 128, :], [128, 2 * D_CONV], BF16)
                for k in range(4)
            ]
            wxp_sb = [
                load_const(f"wxp{k}", wxp_d[k * 128 : (k + 1) * 128, :], [128, D_MAMBA], BF16)
                for k in range(4)
            ]
            wdt_sb = [
                load_const(f"wdt{k}", wdt_d[k * 128 : (k + 1) * 128, :], [128, D_MAMBA], BF16)
                for k in range(4)
            ]
            wbc_sb = [
                load_const(f"wbc{k}", wbc_d[k * 128 : (k + 1) * 128, :], [128, 2 * DSTATE], BF16)
                for k in range(4)
            ]
            wssm_sb = [
                load_const(f"wssm{k}", wssm_d[k * 128 : (k + 1) * 128, :], [128, D_MAMBA], BF16)
                for k in range(2)
            ]
            wop_sb = [
                load_const(f"wop{k}", wop_d[k * 128 : (k + 1) * 128, :], [128, D_MODEL], BF16)
                for k in range(4)
            ]
            w2_sb = [
                load_const(f"w2_{k}", w2_d[k * 128 : (k + 1) * 128, :], [128, D_MODEL], BF16)
                for k in range(NKF)
            ]
            w1_sb = [
                load_const(f"w1_{k}", w1_d[k * 128 : (k + 1) * 128, :], [128, FFN], BF16)
                for k in range(4)
            ]
            w3_sb = [
                load_const(f"w3_{k}", w3_d[k * 128 : (k + 1) * 128, :], [128, FFN], BF16)
                for k in range(4)
            ]
            avec = [
                load_const(f"avec{m}", avec_d[m * 128 : (m + 1) * 128, :], [128, 1])
                for m in range(2)
            ]
            dtb = [
                load_const(f"dtb{m}", dtb_d[m * 128 : (m + 1) * 128, :], [128, 1])
                for m in range(2)
            ]
            dvec = [
                load_const(f"dvec{m}", dvec_d[m * 128 : (m + 1) * 128, :], [128, 1])
                for m in range(2)
            ]
            convb = [
                load_const(f"convb{m}", convb_d[m * 128 : (m + 1) * 128, :], [128, 1])
                for m in range(2)
            ]
            convw = [
                load_const(f"convw{m}", convw_d[m * 128 : (m + 1) * 128, :], [128, KCONV])
                for m in range(2)
            ]
            mask2 = load_const("mask2", mask2_d[:, :], [128, 2])
            sel2 = load_const("sel2", sel2_d[:, :], [2, 128])
            ident = load_const("ident", ident_d[:, :], [128, 128], BF16)

            eps_sb = cp.tile([128, 1], F32, name="eps_sb", tag="eps_sb")
            nc.vector.memset(eps_sb, EPS)
            one_sb = cp.tile([128, 1], F32, name="one_sb", tag="one_sb")
            nc.vector.memset(one_sb, 1.0)

            # ---------------- persistent cross-chunk state
            h_st = [sp.tile([128, 1], F32, name=f"hst{m}", tag=f"hst{m}") for m in range(2)]
            u_halo = [sp.tile([128, 2], F32, name=f"uhalo{m}", tag=f"uhalo{m}") for m in range(2)]
            for m in range(2):
                nc.vector.memset(h_st[m], 0.0)
                nc.vector.memset(u_halo[m], 0.0)

            # ---------------- per-iteration state carried across the skew
            prev = None  # dict with chunk i-1 leftovers

            def load_chunk(i):
                row0 = i * C
                x_nat, v_nat = [], []
                for s in range(NSUB):
                    xt = pio.tile([128, D_MODEL], F32, name="xnat", tag="xnat", bufs=9)
                    nc.sync.dma_start(
                        out=xt, in_=x_d[row0 + s * 128 : row0 + (s + 1) * 128, :]
                    )
                    x_nat.append(xt)
                    vt = pio.tile([128, D_MODEL], F32, name="vnat", tag="vnat", bufs=5)
                    nc.sync.dma_start(
                        out=vt, in_=v_d[row0 + s * 128 : row0 + (s + 1) * 128, :]
                    )
                    v_nat.append(vt)
                return x_nat, v_nat

            def rms_squares(src_tiles, tag):
                """scalar Square + accum -> per-token sum of squares [128,1]x NSUB"""
                ssqs = []
                for s in range(NSUB):
                    scr = pact.tile([128, D_MODEL], F32, name="sqscr", tag="sqscr", bufs=1)
                    ssq = pact.tile([128, 1], F32, name="ssq", tag=tag, bufs=2 * NSUB)
                    nc.scalar.activation(out=scr, in_=src_tiles[s], func=AF.Square, accum_out=ssq)
                    ssqs.append(ssq)
                return ssqs

            def rms_finish(ssqs, tag):
                """scalar: r = exp(-0.5*ln(ms/D + eps))  [nlexp set]"""
                rs = []
                for s in range(NSUB):
                    r = pact.tile([128, 1], F32, name="rr", tag=tag, bufs=2 * NSUB)
                    nc.scalar.activation(
                        out=r, in_=ssqs[s], func=AF.Ln, scale=1.0 / D_MODEL, bias=eps_sb
                    )
                    nc.scalar.activation(out=r, in_=r, func=AF.Exp, scale=-0.5)
                    rs.append(r)
                return rs

            def rms_apply(src_tiles, rs, tag):
                """DVE: xn = x * r -> bf16"""
                outs = []
                for s in range(NSUB):
                    xn = pact.tile([128, D_MODEL], BF16, name=tag, tag=tag, bufs=NSUB + 1)
                    nc.vector.tensor_scalar(
                        out=xn, in0=src_tiles[s], scalar1=rs[s], scalar2=None, op0=OP.mult
                    )
                    outs.append(xn)
                return outs

            def transpose_tiles(nat_tiles, dst_tag):
                """NSUB x [128,D_MODEL](bf16) natural -> 4 x [128,C](bf16) transposed."""
                outT = []
                for d in range(4):
                    ps = psT.tile([128, C], BF16, name="psTt", tag="psT")
                    for s in range(NSUB):
                        nc.tensor.transpose(
                            out=ps[:, s * 128 : (s + 1) * 128],
                            in_=nat_tiles[s][:, d * 128 : (d + 1) * 128],
                            identity=ident,
                        )
                    t = pact.tile([128, C], BF16, name=dst_tag, tag=dst_tag, bufs=5)
                    nc.vector.tensor_copy(out=t, in_=ps)
                    outT.append(t)
                return outT

            # ================================================ main loop (skewed)
            for i in range(NCH + 1):
                cur = None
                if i < NCH:
                    cur = {}
                    # ---- DMA in + rmsnorm1 (squares emitted in prev silu block
                    # for i>0; here for i==0)
                    x_nat, v_nat = load_chunk(i)
                    cur["x_nat"], cur["v_nat"] = x_nat, v_nat

                    # ======== NLEXP scalar block for iteration i ========
                    # (Square is in every table set: placing it here costs no
                    # table load, and it only depends on the x DMA -- so the
                    # xnT critical path never waits on chunk i-1's mixer.)
                    ssq1 = rms_squares(x_nat, "ssq1")
                    r1 = rms_finish(ssq1, "r1")
                    xn = rms_apply(x_nat, r1, "xn")
                    cur["xn"] = xn

                # rmsnorm2 of chunk i-1 (x2 lives in x_nat tiles of i-1)
                if prev is not None:
                    ssq2 = rms_squares(prev["x_nat"], "ssq2")
                    r2 = rms_finish(ssq2, "r2")
                    n2 = rms_apply(prev["x_nat"], r2, "n2")
                    prev["n2"] = n2

                if cur is not None:
                    # ---- PE: transposes + projections
                    xnT = transpose_tiles(cur["xn"], "xnT")
                    cur["xnT"] = xnT

                    # conv input proj u (m=0,1)  [gate g deferred to later]
                    u_ps = []
                    for m in range(2):
                        ps = psP.tile([128, C], F32, name="psPu", tag="psP")
                        for k in range(4):
                            mm(ps, wconv_sb[k][:, m * 128 : (m + 1) * 128], xnT[k],
                               start=(k == 0), stop=(k == 3))
                        u_ps.append(ps)
                    # x_ssm
                    xssm_ps = []
                    for m in range(2):
                        ps = psP.tile([128, C], F32, name="psPxs", tag="psP")
                        for k in range(4):
                            mm(ps, wxp_sb[k][:, m * 128 : (m + 1) * 128], xnT[k],
                               start=(k == 0), stop=(k == 3))
                        xssm_ps.append(ps)
                    # dt raw
                    dt_ps = []
                    for m in range(2):
                        ps = psP.tile([128, C], F32, name="psPdt", tag="psP")
                        for k in range(4):
                            mm(ps, wdt_sb[k][:, m * 128 : (m + 1) * 128], xnT[k],
                               start=(k == 0), stop=(k == 3))
                        dt_ps.append(ps)
                    # B/C merged [128 out rows: 0-63 B, 64-127 C]
                    bc_ps = psP.tile([128, C], F32, name="psPbc", tag="psP")
                    for k in range(4):
                        mm(bc_ps, wbc_sb[k], xnT[k], start=(k == 0), stop=(k == 3))
                    # conv gate g (m=0,1) -- late alloc (consumed in silu block)
                    g_ps = []
                    for m in range(2):
                        ps = psP.tile([128, C], F32, name="psPg", tag="psP")
                        for k in range(4):
                            mm(ps, wconv_sb[k][:, (2 + m) * 128 : (3 + m) * 128], xnT[k],
                               start=(k == 0), stop=(k == 3))
                        g_ps.append(ps)

                    # ---- DVE: conv u -> SBUF with halo
                    u_ext = []
                    for m in range(2):
                        ue = pact.tile([128, C + 2], F32, name="uext", tag="uext", bufs=2)
                        nc.vector.tensor_copy(out=ue[:, 2 : C + 2], in_=u_ps[m])
                        nc.vector.tensor_copy(out=ue[:, 0:2], in_=u_halo[m])
                        nc.vector.tensor_copy(out=u_halo[m], in_=ue[:, C : C + 2])
                        u_ext.append(ue)

                    # ---- scalar: xssm copies to SBUF (frees PSUM)
                    xssm_sb = []
                    for m in range(2):
                        xs = pact.tile([128, C], F32, name="xssm", tag="xssm", bufs=2)
                        nc.scalar.copy(out=xs, in_=xssm_ps[m])
                        xssm_sb.append(xs)

                    # ---- scalar: dt softplus in-place in PSUM [nlexp]
                    dtf = []
                    for m in range(2):
                        nc.scalar.activation(out=dt_ps[m], in_=dt_ps[m], func=AF.Exp, bias=dtb[m])
                        nc.scalar.activation(out=dt_ps[m], in_=dt_ps[m], func=AF.Ln, bias=one_sb)
                        df = pact.tile([128, C], F32, name="dtf", tag="dtf", bufs=2)
                        nc.vector.tensor_scalar(
                            out=df, in0=dt_ps[m], scalar1=1e-4, scalar2=0.1,
                            op0=OP.max, op1=OP.min,
                        )
                        dtf.append(df)
                    # decay = exp(A*dt)
                    decay = []
                    for m in range(2):
                        dc = pact.tile([128, C], F32, name="dec", tag="dec", bufs=2)
                        nc.scalar.activation(out=dc, in_=dtf[m], func=AF.Exp, scale=avec[m])
                        decay.append(dc)

                    # scalar part of B/C row norm (sq for partition-reduce; bm copy)
                    sqbc = pact.tile([128, C], F32, name="sqbc", tag="sqbc", bufs=2)
                    nc.scalar.activation(out=sqbc, in_=bc_ps, func=AF.Square)
                    bm_sb = pact.tile([128, C], F32, name="bmsb", tag="bmsb", bufs=2)
                    nc.scalar.copy(out=bm_sb, in_=bc_ps)

                # ---- PE: nT transposes for chunk i-1 (needs n2 from nlexp above;
                # keeps PE busy while the scalar engine works through the B/C chain)
                if prev is not None:
                    nT = transpose_tiles(prev["n2"], "nT")
                    prev["nT"] = nT

                if cur is not None:
                    # ---- B/C row norm: r = exp(-0.5*relu(ln(s)))  (== min(1/sqrt(s),1))
                    sum2_ps = psP.tile([128, C], F32, name="psPs2", tag="psP")
                    mm(sum2_ps[0:2, :], mask2, sqbc, start=True, stop=True)
                    rbc = pact.tile([2, C], F32, name="rbc", tag="rbc", bufs=2)
                    nc.scalar.activation(out=rbc, in_=sum2_ps[0:2, :], func=AF.Ln)
                    nc.scalar.activation(out=rbc, in_=rbc, func=AF.Relu)
                    nc.scalar.activation(out=rbc, in_=rbc, func=AF.Exp, scale=-0.5)
                    sel_ps = psP.tile([128, C], F32, name="psPsel", tag="psP")
                    mm(sel_ps, sel2, rbc, start=True, stop=True)
                    bcn = pact.tile([128, C], F32, name="bcn", tag="bcn", bufs=2)
                    nc.vector.tensor_mul(out=bcn, in0=bm_sb, in1=sel_ps)
                    b128 = pact.tile([128, C], F32, name="b128", tag="b128", bufs=2)
                    c128 = pact.tile([128, C], F32, name="c128", tag="c128", bufs=2)
                    nc.sync.dma_start(out=b128[0:64, :], in_=bcn[0:64, :])
                    nc.sync.dma_start(out=b128[64:128, :], in_=bcn[0:64, :])
                    nc.sync.dma_start(out=c128[0:64, :], in_=bcn[64:128, :])
                    nc.sync.dma_start(out=c128[64:128, :], in_=bcn[64:128, :])

                # ======== SILU scalar block ========
                if cur is not None:
                    # conv gate silu (frees g psum)
                    gs = []
                    for m in range(2):
                        g = pact.tile([128, C], F32, name="gs", tag="gs", bufs=2)
                        nc.scalar.activation(out=g, in_=g_ps[m], func=AF.Silu)
                        gs.append(g)

                    # ---- DVE: depthwise conv FMA chain + gate
                    conv_out = []
                    for m in range(2):
                        cc = pact.tile([128, C], F32, name="cc", tag="cc", bufs=2)
                        nc.vector.tensor_scalar(
                            out=cc, in0=u_ext[m][:, 0:C], scalar1=convw[m][:, 0:1],
                            scalar2=convb[m], op0=OP.mult, op1=OP.add,
                        )
                        for kk in (1, 2):
                            nc.vector.scalar_tensor_tensor(
                                out=cc, in0=u_ext[m][:, kk : C + kk],
                                scalar=convw[m][:, kk : kk + 1], in1=cc,
                                op0=OP.mult, op1=OP.add,
                            )
                        co = pact.tile([128, C], BF16, name="convout", tag="convout", bufs=3)
                        nc.vector.tensor_mul(out=co, in0=cc, in1=gs[m])
                        conv_out.append(co)

                    # ---- scan chain (DVE + gpsimd)
                    yT = []
                    for m in range(2):
                        tmp = pact.tile([128, C], F32, name="tmp", tag="tmp", bufs=2)
                        nc.vector.tensor_mul(out=tmp, in0=dtf[m], in1=xssm_sb[m])
                        inp = pact.tile([128, C], F32, name="inp", tag="inp", bufs=2)
                        nc.gpsimd.tensor_mul(out=inp, in0=tmp, in1=b128)
                        hs = pact.tile([128, C], F32, name="hs", tag="hs", bufs=2)
                        nc.vector.tensor_tensor_scan(
                            out=hs, data0=decay[m], data1=inp, initial=h_st[m],
                            op0=OP.mult, op1=OP.add,
                        )
                        nc.vector.tensor_copy(out=h_st[m], in_=hs[:, C - 1 : C])
                        hc = pact.tile([128, C], F32, name="hc", tag="hc", bufs=2)
                        nc.gpsimd.tensor_mul(out=hc, in0=hs, in1=c128)
                        yt = pact.tile([128, C], BF16, name="yt", tag="yt", bufs=2)
                        nc.vector.scalar_tensor_tensor(
                            out=yt, in0=xssm_sb[m], scalar=dvec[m], in1=hc,
                            op0=OP.mult, op1=OP.add,
                        )
                        yT.append(yt)
                    cur["yT"] = yT

                # ---- FFN of chunk i-1: w1/w3 matmuls + silu + gate + w2 (pass li01)
                if prev is not None:
                    nT = prev["nT"]
                    psf01 = [psF.tile([128, D_MODEL], F32, name="psf", tag="psF") for _ in range(2)]
                    h_sbs = []
                    for kf in range(NKF):
                        pa = psP.tile([128, C], F32, name="psPa", tag="psP")
                        for k in range(4):
                            mm(pa, w1_sb[k][:, kf * 128 : (kf + 1) * 128], nT[k],
                               start=(k == 0), stop=(k == 3))
                        pb = psP.tile([128, C], F32, name="psPb", tag="psP")
                        for k in range(4):
                            mm(pb, w3_sb[k][:, kf * 128 : (kf + 1) * 128], nT[k],
                               start=(k == 0), stop=(k == 3))
                        h_t = pffn.tile([128, C], F32, name="ht", tag="ht", bufs=2)
                        nc.scalar.activation(out=h_t, in_=pa, func=AF.Silu)
                        h_sb = pffn.tile([128, C], BF16, name="hsb", tag="hsb", bufs=NKF)
                        nc.vector.tensor_mul(out=h_sb, in0=pb, in1=h_t)
                        h_sbs.append(h_sb)
                        for li in range(2):
                            mm(psf01[li], h_sb[:, li * 128 : (li + 1) * 128], w2_sb[kf],
                               start=(kf == 0), stop=(kf == NKF - 1))
                    # residual + DMA out for token subtiles 0,1 (frees psF slots
                    # so pass li23 can allocate while PE runs ssm_out/mixer)
                    for li in range(2):
                        xt = prev["x_nat"][li]
                        nc.vector.tensor_add(out=xt, in0=xt, in1=psf01[li])
                        nc.sync.dma_start(
                            out=xo_d[(i - 1) * C + li * 128 : (i - 1) * C + (li + 1) * 128, :],
                            in_=xt,
                        )
                    prev["h_sbs"] = h_sbs

                # ---- PE: ssm_out + mixer for chunk i (scan done by now)
                if cur is not None:
                    y2T = []
                    for m in range(2):
                        ps = psP.tile([128, C], F32, name="psPy2", tag="psP")
                        for k in range(2):
                            mm(ps, wssm_sb[k][:, m * 128 : (m + 1) * 128], cur["yT"][k],
                               start=(k == 0), stop=(k == 1))
                        y2 = pact.tile([128, C], BF16, name="y2", tag="y2", bufs=2)
                        nc.vector.tensor_copy(out=y2, in_=ps)
                        y2T.append(y2)

                    mix_lhsT = [conv_out[0], conv_out[1], y2T[0], y2T[1]]
                    for li in range(NSUB):
                        ps = psT.tile([128, D_MODEL], F32, name="psTm", tag="psT")
                        for k in range(4):
                            mm(ps, mix_lhsT[k][:, li * 128 : (li + 1) * 128], wop_sb[k],
                               start=(k == 0), stop=(k == 3))
                        # v_new = beta*v + mixer  (in-place into v tile)
                        vt = cur["v_nat"][li]
                        nc.vector.scalar_tensor_tensor(
                            out=vt, in0=vt, scalar=beta, in1=ps,
                            op0=OP.mult, op1=OP.add,
                        )
                        nc.sync.dma_start(
                            out=vo_d[i * C + li * 128 : i * C + (li + 1) * 128, :], in_=vt
                        )
                        # x2 = x + v_new  (in-place into x tile, gpsimd)
                        xt = cur["x_nat"][li]
                        nc.gpsimd.tensor_add(out=xt, in0=xt, in1=vt)

                # ---- FFN pass li23 + final residual + DMA out for chunk i-1
                if prev is not None:
                    psf23 = [psF.tile([128, D_MODEL], F32, name="psf", tag="psF") for _ in range(2)]
                    for kf in range(NKF):
                        for li in range(2):
                            mm(psf23[li], prev["h_sbs"][kf][:, (2 + li) * 128 : (3 + li) * 128],
                               w2_sb[kf], start=(kf == 0), stop=(kf == NKF - 1))
                    for li in range(2, NSUB):
                        xt = prev["x_nat"][li]
                        nc.vector.tensor_add(out=xt, in0=xt, in1=psf23[li - 2])
                        nc.sync.dma_start(
                            out=xo_d[(i - 1) * C + li * 128 : (i - 1) * C + (li + 1) * 128, :],
                            in_=xt,
                        )

                prev = cur

    if split:
        split_waits(nc)
    return nc


# ---------------------------------------------------------------- host glue
def prep_weights(inputs):
    """Host-side preprocessing: fold norm weights into matmul weights,
    precompute A = -exp(A_log), beta, and small constant matrices."""
    f = lambda a: np.asarray(a, dtype=np.float32)
    bf = lambda a: np.ascontiguousarray(np.asarray(a, dtype=np.float32).astype(ml_dtypes.bfloat16))
    pre_w = f(inputs["pre_norm_w"])[:, None]
    ffn_w = f(inputs["ffn_norm_w"])[:, None]
    A = -np.exp(f(inputs["A_log"]).reshape(-1))
    mask2 = np.zeros((128, 2), np.float32)
    mask2[0:64, 0] = 1.0
    mask2[64:128, 1] = 1.0
    sel2 = np.zeros((2, 128), np.float32)
    sel2[0, 0:64] = 1.0
    sel2[1, 64:128] = 1.0
    beta = float(1.0 / (1.0 + np.exp(-f(inputs["log_beta"]))))
    wbc = np.concatenate([pre_w * f(inputs["B_w"]), pre_w * f(inputs["C_w"])], axis=1)
    w = {
        "w_conv": bf(pre_w * f(inputs["conv_in_w"])),
        "w_xproj": bf(pre_w * f(inputs["x_proj_w"])),
        "w_dt": bf(pre_w * f(inputs["dt_w"])),
        "w_bc": bf(wbc),
        "w_ssmout": bf(f(inputs["ssm_out_w"])),
        "w_outproj": bf(f(inputs["out_proj_w"])),
        "w1": bf(ffn_w * f(inputs["w1"])),
        "w3": bf(ffn_w * f(inputs["w3"])),
        "w2": bf(f(inputs["w2"])),
        "a_vec": A[:, None].copy(),
        "dtb_vec": f(inputs["dt_b"])[:, None].copy(),
        "d_vec": f(inputs["D"])[:, None].copy(),
        "convb_vec": f(inputs["conv_dw_b"])[:, None].copy(),
        "convw": np.ascontiguousarray(f(inputs["conv_dw_w"])),
        "mask2": mask2,
        "sel2": sel2,
        "ident": np.ascontiguousarray(np.eye(128, dtype=np.float32).astype(ml_dtypes.bfloat16)),
    }
    return w, beta


CHUNK = 512

_PROG_CACHE = {}


def kernel(**inputs):
    """Full-input entry point: shard batch over the 8 NeuronCores (one batch
    element per core; the scan state is per-(batch,channel) so this is
    embarrassingly parallel), run the Bass program SPMD, regather."""
    w, beta = prep_weights(inputs)
    x = np.asarray(inputs["x"], np.float32)
    v = np.asarray(inputs["velocity"], np.float32)
    n_cores, L, _ = x.shape
    key = (L, CHUNK, beta)
    if key not in _PROG_CACHE:
        _PROG_CACHE[key] = build_program(L, CHUNK, beta)
    nc = _PROG_CACHE[key]
    in_maps = []
    for b in range(n_cores):
        m = dict(w)
        m["x"] = np.ascontiguousarray(x[b])
        m["v"] = np.ascontiguousarray(v[b])
        in_maps.append(m)
    res = run_bass_kernel_spmd(nc, in_maps, core_ids=list(range(n_cores)))
    x_out = np.stack([res.results[b]["x_out"] for b in range(n_cores)])
    v_out = np.stack([res.results[b]["v_out"] for b in range(n_cores)])
    return (x_out, v_out)


# revision 15
# speedup vs baseline: 1.5833x; 1.2065x over previous
"""CoreHybridBlock Trainium2 kernel: builder + host glue (v2).

Per-core program (one batch element per core), chunked over tokens (C=512):
  natural layout = [token(part), feature(free)], transposed = [feature(part), token(free)]

  Pipeline is skewed one chunk for the FFN so the scalar engine needs only
  two activation-table switches per iteration (nlexp set <-> silu set):

  iteration i:
    nlexp block: rmsnorm1(i) rsqrt via exp(-ln/2); rmsnorm2(i-1);
                 dt softplus via ln(1+exp); decay exp; B/C rownorm via
                 exp(-relu(ln)/2)  [clip(norm,1) done in log space]
    silu block:  conv gate silu(i); ffn silu(i-1); rmsnorm1-square(i+1)
    PE: xnT(i) transposes, projections(i), nT(i-1) transposes,
        ffn(i-1) w1/w3 + w2(pass li01), ssm_out(i), mixer(i), w2(pass li23)
    DVE: copies/casts, conv FMA chain, scan, gate muls, residual stts
    GpSimd: scan input/output muls, x2 residual add (SBUF-only fp32)

  All matmul operands bf16 (fp32 accumulation in PSUM); residual stream,
  scan, and scalar chains stay fp32.
"""

import ml_dtypes
import numpy as np
import bass_rust
import concourse.bass as bass
import concourse.tile as tile
from concourse import mybir
from concourse.bass_utils import run_bass_kernel_spmd

F32 = mybir.dt.float32
BF16 = mybir.dt.bfloat16
F8 = mybir.dt.float8e4
DR = mybir.MatmulPerfMode.DoubleRow
SFF = 32.0  # fp8 ffn weight scale (h absmax ~2.9 -> 32*h ~ 92 < 240)
AF = mybir.ActivationFunctionType
OP = mybir.AluOpType

D_MODEL, D_CONV, D_MAMBA = 512, 256, 256
DSTATE, N_HEADS, KCONV, FFN = 64, 4, 3, 2048
EPS = 1e-6
NKF = FFN // 128  # 16


# ---------------------------------------------------------------- wait split
def split_waits(nc, max_w=1):
    """walrus in this container rejects >~1 sync wait per instruction on some
    instruction types (the Tile end-drain carries one wait per live
    semaphore).  Hoist excess waits onto same-engine NoOps placed before the
    offending instruction."""
    cnt = 0
    for f in nc.m.functions:
        for bb in f.blocks:
            new_list = []
            changed = False
            for inst in bb.instructions:
                si = inst.sync_info
                waits = list(si.on_wait) if si is not None and si.on_wait else []
                if len(waits) > max_w:
                    changed = True
                    extra = waits[max_w:]
                    si.on_wait = waits[:max_w]
                    for j in range(0, len(extra), max_w):
                        cnt += 1
                        nop = bass_rust.InstNoOp(
                            name=f"I-waitsplit-{cnt}", ins=[], outs=[]
                        )
                        nop.engine = inst.engine
                        nop.sync_info = bass_rust.SyncInfo(
                            on_wait=extra[j : j + max_w], on_update=[]
                        )
                        new_list.append(nop)
                new_list.append(inst)
            if changed:
                bb.instructions = new_list
    return cnt


# ---------------------------------------------------------------- program
def build_program(L, C, beta, split=True):
    """One-core program; SPMD over 8 cores with different x/v slices."""
    NCH = L // C
    NSUB = C // 128  # token-subtiles per chunk (natural layout)
    nc = bass.Bass()

    # ---- dram I/O
    x_d = nc.dram_tensor("x", [L, D_MODEL], F32, kind="ExternalInput")
    v_d = nc.dram_tensor("v", [L, D_MODEL], F32, kind="ExternalInput")
    wconv_d = nc.dram_tensor("w_conv", [D_MODEL, 2 * D_CONV], BF16, kind="ExternalInput")
    wxp_d = nc.dram_tensor("w_xproj", [D_MODEL, D_MAMBA], BF16, kind="ExternalInput")
    wdt_d = nc.dram_tensor("w_dt", [D_MODEL, D_MAMBA], BF16, kind="ExternalInput")
    wbc_d = nc.dram_tensor("w_bc", [D_MODEL, 2 * DSTATE], BF16, kind="ExternalInput")
    wssm_d = nc.dram_tensor("w_ssmout", [D_MAMBA, D_MAMBA], BF16, kind="ExternalInput")
    wop_d = nc.dram_tensor("w_outproj", [D_MODEL, D_MODEL], BF16, kind="ExternalInput")
    w1_d = nc.dram_tensor("w1", [128, 4, FFN], F8, kind="ExternalInput")
    w3_d = nc.dram_tensor("w3", [128, 4, FFN], F8, kind="ExternalInput")
    w2_d = nc.dram_tensor("w2", [128, NKF, D_MODEL], F8, kind="ExternalInput")
    avec_d = nc.dram_tensor("a_vec", [D_MAMBA, 1], F32, kind="ExternalInput")
    dtb_d = nc.dram_tensor("dtb_vec", [D_MAMBA, 1], F32, kind="ExternalInput")
    dvec_d = nc.dram_tensor("d_vec", [D_MAMBA, 1], F32, kind="ExternalInput")
    convb_d = nc.dram_tensor("convb_vec", [D_CONV, 1], F32, kind="ExternalInput")
    convw_d = nc.dram_tensor("convw", [D_CONV, KCONV], F32, kind="ExternalInput")
    mask2_d = nc.dram_tensor("mask2", [128, 2], F32, kind="ExternalInput")
    sel2_d = nc.dram_tensor("sel2", [2, 128], F32, kind="ExternalInput")
    ident_d = nc.dram_tensor("ident", [128, 128], BF16, kind="ExternalInput")

    xo_d = nc.dram_tensor("x_out", [L, D_MODEL], F32, kind="ExternalOutput")
    vo_d = nc.dram_tensor("v_out", [L, D_MODEL], F32, kind="ExternalOutput")

    with tile.TileContext(nc) as tc:
        with (
            tc.tile_pool(name="consts", bufs=1) as cp,
            tc.tile_pool(name="state", bufs=1) as sp,
            tc.tile_pool(name="io", bufs=2) as pio,
            tc.tile_pool(name="act", bufs=2) as pact,
            tc.tile_pool(name="ffn", bufs=2) as pffn,
            tc.tile_pool(name="psT", bufs=2, space="PSUM") as psT,
            tc.tile_pool(name="psP", bufs=4, space="PSUM") as psP,
            tc.tile_pool(name="psF", bufs=2, space="PSUM") as psF,
        ):
            def mm(out, lhsT, rhs, start, stop, pm=None):
                nc.tensor.matmul(out=out, lhsT=lhsT, rhs=rhs, start=start, stop=stop, perf_mode=pm)

            # ---------------- constants / weights resident in SBUF
            def load_const(name, dram_ap, shape, dt=F32):
                t = cp.tile(shape, dt, name=name, tag=name)
                nc.sync.dma_start(out=t, in_=dram_ap)
                return t

            wconv_sb = [
                load_const(f"wconv{k}", wconv_d[k * 128 : (k + 1) * 128, :], [128, 2 * D_CONV], BF16)
                for k in range(4)
            ]
            wxp_sb = [
                load_const(f"wxp{k}", wxp_d[k * 128 : (k + 1) * 128, :], [128, D_MAMBA], BF16)
                for k in range(4)
            ]
            wdt_sb = [
                load_const(f"wdt{k}", wdt_d[k * 128 : (k + 1) * 128, :], [128, D_MAMBA], BF16)
                for k in range(4)
            ]
            wbc_sb = [
                load_const(f"wbc{k}", wbc_d[k * 128 : (k + 1) * 128, :], [128, 2 * DSTATE], BF16)
                for k in range(4)
            ]
            wssm_sb = [
                load_const(f"wssm{k}", wssm_d[k * 128 : (k + 1) * 128, :], [128, D_MAMBA], BF16)
                for k in range(2)
            ]
            wop_sb = [
                load_const(f"wop{k}", wop_d[k * 128 : (k + 1) * 128, :], [128, D_MODEL], BF16)
                for k in range(4)
            ]
            w1_sb = load_const("w1_sb", w1_d[:, :, :], [128, 4, FFN], F8)
            w3_sb = load_const("w3_sb", w3_d[:, :, :], [128, 4, FFN], F8)
            w2_sb = load_const("w2_sb", w2_d[:, :, :], [128, NKF, D_MODEL], F8)
            avec = [
                load_const(f"avec{m}", avec_d[m * 128 : (m + 1) * 128, :], [128, 1])
                for m in range(2)
            ]
            dtb = [
                load_const(f"dtb{m}", dtb_d[m * 128 : (m + 1) * 128, :], [128, 1])
                for m in range(2)
            ]
            dvec = [
                load_const(f"dvec{m}", dvec_d[m * 128 : (m + 1) * 128, :], [128, 1])
                for m in range(2)
            ]
            convb = [
                load_const(f"convb{m}", convb_d[m * 128 : (m + 1) * 128, :], [128, 1])
                for m in range(2)
            ]
            convw = [
                load_const(f"convw{m}", convw_d[m * 128 : (m + 1) * 128, :], [128, KCONV])
                for m in range(2)
            ]
            mask2 = load_const("mask2", mask2_d[:, :], [128, 2])
            sel2 = load_const("sel2", sel2_d[:, :], [2, 128])
            ident = load_const("ident", ident_d[:, :], [128, 128], BF16)

            eps_sb = cp.tile([128, 1], F32, name="eps_sb", tag="eps_sb")
            nc.vector.memset(eps_sb, EPS)
            one_sb = cp.tile([128, 1], F32, name="one_sb", tag="one_sb")
            nc.vector.memset(one_sb, 1.0)

            # ---------------- persistent cross-chunk state
            h_st = [sp.tile([128, 1], F32, name=f"hst{m}", tag=f"hst{m}") for m in range(2)]
            u_halo = [sp.tile([128, 2], F32, name=f"uhalo{m}", tag=f"uhalo{m}") for m in range(2)]
            for m in range(2):
                nc.vector.memset(h_st[m], 0.0)
                nc.vector.memset(u_halo[m], 0.0)

            # ---------------- per-iteration state carried across the skew
            prev = None  # dict with chunk i-1 leftovers

            def load_chunk(i):
                row0 = i * C
                x_nat, v_nat = [], []
                for s in range(NSUB):
                    xt = pio.tile([128, D_MODEL], F32, name="xnat", tag="xnat", bufs=9)
                    nc.sync.dma_start(
                        out=xt, in_=x_d[row0 + s * 128 : row0 + (s + 1) * 128, :]
                    )
                    x_nat.append(xt)
                    vt = pio.tile([128, D_MODEL], F32, name="vnat", tag="vnat", bufs=5)
                    nc.sync.dma_start(
                        out=vt, in_=v_d[row0 + s * 128 : row0 + (s + 1) * 128, :]
                    )
                    v_nat.append(vt)
                return x_nat, v_nat

            def rms_squares(src_tiles, tag):
                """scalar Square + accum -> per-token sum of squares [128,1]x NSUB"""
                ssqs = []
                for s in range(NSUB):
                    scr = pact.tile([128, D_MODEL], F32, name="sqscr", tag="sqscr", bufs=1)
                    ssq = pact.tile([128, 1], F32, name="ssq", tag=tag, bufs=2 * NSUB)
                    nc.scalar.activation(out=scr, in_=src_tiles[s], func=AF.Square, accum_out=ssq)
                    ssqs.append(ssq)
                return ssqs

            def rms_finish(ssqs, tag):
                """scalar: r = exp(-0.5*ln(ms/D + eps))  [nlexp set]"""
                rs = []
                for s in range(NSUB):
                    r = pact.tile([128, 1], F32, name="rr", tag=tag, bufs=2 * NSUB)
                    nc.scalar.activation(
                        out=r, in_=ssqs[s], func=AF.Ln, scale=1.0 / D_MODEL, bias=eps_sb
                    )
                    nc.scalar.activation(out=r, in_=r, func=AF.Exp, scale=-0.5)
                    rs.append(r)
                return rs

            def rms_apply(src_tiles, rs, tag):
                """DVE: xn = x * r -> bf16"""
                outs = []
                for s in range(NSUB):
                    xn = pact.tile([128, D_MODEL], BF16, name=tag, tag=tag, bufs=NSUB + 1)
                    nc.vector.tensor_scalar(
                        out=xn, in0=src_tiles[s], scalar1=rs[s], scalar2=None, op0=OP.mult
                    )
                    outs.append(xn)
                return outs

            def transpose_tiles(nat_tiles, dst_tag):
                """NSUB x [128,D_MODEL](bf16) natural -> 4 x [128,C](bf16) transposed."""
                outT = []
                for d in range(4):
                    ps = psT.tile([128, C], BF16, name="psTt", tag="psT")
                    for s in range(NSUB):
                        nc.tensor.transpose(
                            out=ps[:, s * 128 : (s + 1) * 128],
                            in_=nat_tiles[s][:, d * 128 : (d + 1) * 128],
                            identity=ident,
                        )
                    t = pact.tile([128, C], BF16, name=dst_tag, tag=dst_tag, bufs=5)
                    nc.vector.tensor_copy(out=t, in_=ps)
                    outT.append(t)
                return outT

            # ================================================ main loop (skewed)
            for i in range(NCH + 1):
                cur = None
                if i < NCH:
                    cur = {}
                    # ---- DMA in + rmsnorm1 (squares emitted in prev silu block
                    # for i>0; here for i==0)
                    x_nat, v_nat = load_chunk(i)
                    cur["x_nat"], cur["v_nat"] = x_nat, v_nat

                    # ======== NLEXP scalar block for iteration i ========
                    # (Square is in every table set: placing it here costs no
                    # table load, and it only depends on the x DMA -- so the
                    # xnT critical path never waits on chunk i-1's mixer.)
                    ssq1 = rms_squares(x_nat, "ssq1")
                    r1 = rms_finish(ssq1, "r1")
                    xn = rms_apply(x_nat, r1, "xn")
                    cur["xn"] = xn

                # rmsnorm2 of chunk i-1 (x2 lives in x_nat tiles of i-1)
                if prev is not None:
                    ssq2 = rms_squares(prev["x_nat"], "ssq2")
                    r2 = rms_finish(ssq2, "r2")
                    n2 = rms_apply(prev["x_nat"], r2, "n2")
                    prev["n2"] = n2

                if cur is not None:
                    # ---- PE: transposes + projections
                    xnT = transpose_tiles(cur["xn"], "xnT")
                    cur["xnT"] = xnT

                    # conv input proj u (m=0,1)  [gate g deferred to later]
                    u_ps = []
                    for m in range(2):
                        ps = psP.tile([128, C], F32, name="psPu", tag="psP")
                        for k in range(4):
                            mm(ps, wconv_sb[k][:, m * 128 : (m + 1) * 128], xnT[k],
                               start=(k == 0), stop=(k == 3))
                        u_ps.append(ps)
                    # x_ssm
                    xssm_ps = []
                    for m in range(2):
                        ps = psP.tile([128, C], F32, name="psPxs", tag="psP")
                        for k in range(4):
                            mm(ps, wxp_sb[k][:, m * 128 : (m + 1) * 128], xnT[k],
                               start=(k == 0), stop=(k == 3))
                        xssm_ps.append(ps)
                    # dt raw
                    dt_ps = []
                    for m in range(2):
                        ps = psP.tile([128, C], F32, name="psPdt", tag="psP")
                        for k in range(4):
                            mm(ps, wdt_sb[k][:, m * 128 : (m + 1) * 128], xnT[k],
                               start=(k == 0), stop=(k == 3))
                        dt_ps.append(ps)
                    # B/C merged [128 out rows: 0-63 B, 64-127 C]
                    bc_ps = psP.tile([128, C], F32, name="psPbc", tag="psP")
                    for k in range(4):
                        mm(bc_ps, wbc_sb[k], xnT[k], start=(k == 0), stop=(k == 3))
                    # conv gate g (m=0,1) -- late alloc (consumed in silu block)
                    g_ps = []
                    for m in range(2):
                        ps = psP.tile([128, C], F32, name="psPg", tag="psP")
                        for k in range(4):
                            mm(ps, wconv_sb[k][:, (2 + m) * 128 : (3 + m) * 128], xnT[k],
                               start=(k == 0), stop=(k == 3))
                        g_ps.append(ps)

                    # ---- DVE: conv u -> SBUF with halo
                    u_ext = []
                    for m in range(2):
                        ue = pact.tile([128, C + 2], F32, name="uext", tag="uext", bufs=2)
                        nc.vector.tensor_copy(out=ue[:, 2 : C + 2], in_=u_ps[m])
                        nc.vector.tensor_copy(out=ue[:, 0:2], in_=u_halo[m])
                        nc.vector.tensor_copy(out=u_halo[m], in_=ue[:, C : C + 2])
                        u_ext.append(ue)

                    # ---- scalar: xssm copies to SBUF (frees PSUM)
                    xssm_sb = []
                    for m in range(2):
                        xs = pact.tile([128, C], F32, name="xssm", tag="xssm", bufs=2)
                        nc.scalar.copy(out=xs, in_=xssm_ps[m])
                        xssm_sb.append(xs)

                    # ---- scalar: dt softplus in-place in PSUM [nlexp]
                    dtf = []
                    for m in range(2):
                        nc.scalar.activation(out=dt_ps[m], in_=dt_ps[m], func=AF.Exp, bias=dtb[m])
                        nc.scalar.activation(out=dt_ps[m], in_=dt_ps[m], func=AF.Ln, bias=one_sb)
                        df = pact.tile([128, C], F32, name="dtf", tag="dtf", bufs=2)
                        nc.vector.tensor_scalar(
                            out=df, in0=dt_ps[m], scalar1=1e-4, scalar2=0.1,
                            op0=OP.max, op1=OP.min,
                        )
                        dtf.append(df)
                    # decay = exp(A*dt)
                    decay = []
                    for m in range(2):
                        dc = pact.tile([128, C], F32, name="dec", tag="dec", bufs=2)
                        nc.scalar.activation(out=dc, in_=dtf[m], func=AF.Exp, scale=avec[m])
                        decay.append(dc)

                    # scalar part of B/C row norm (sq for partition-reduce; bm copy)
                    sqbc = pact.tile([128, C], F32, name="sqbc", tag="sqbc", bufs=2)
                    nc.scalar.activation(out=sqbc, in_=bc_ps, func=AF.Square)
                    bm_sb = pact.tile([128, C], F32, name="bmsb", tag="bmsb", bufs=2)
                    nc.scalar.copy(out=bm_sb, in_=bc_ps)

                # ---- PE: nT transposes for chunk i-1 (needs n2 from nlexp above;
                # keeps PE busy while the scalar engine works through the B/C chain)
                if prev is not None:
                    n8 = pffn.tile([128, 4, C], F8, name="n8", tag="n8", bufs=2)
                    for d in range(4):
                        ps = psT.tile([128, C], BF16, name="psTt", tag="psT")
                        for s in range(NSUB):
                            nc.tensor.transpose(
                                out=ps[:, s * 128 : (s + 1) * 128],
                                in_=prev["n2"][s][:, d * 128 : (d + 1) * 128],
                                identity=ident,
                            )
                        nc.vector.tensor_copy(out=n8[:, d, :], in_=ps)
                    prev["n8"] = n8

                if cur is not None:
                    # ---- B/C row norm: r = exp(-0.5*relu(ln(s)))  (== min(1/sqrt(s),1))
                    sum2_ps = psP.tile([128, C], F32, name="psPs2", tag="psP")
                    mm(sum2_ps[0:2, :], mask2, sqbc, start=True, stop=True)
                    rbc = pact.tile([2, C], F32, name="rbc", tag="rbc", bufs=2)
                    nc.scalar.activation(out=rbc, in_=sum2_ps[0:2, :], func=AF.Ln)
                    nc.scalar.activation(out=rbc, in_=rbc, func=AF.Relu)
                    nc.scalar.activation(out=rbc, in_=rbc, func=AF.Exp, scale=-0.5)
                    sel_ps = psP.tile([128, C], F32, name="psPsel", tag="psP")
                    mm(sel_ps, sel2, rbc, start=True, stop=True)
                    bcn = pact.tile([128, C], F32, name="bcn", tag="bcn", bufs=2)
                    nc.vector.tensor_mul(out=bcn, in0=bm_sb, in1=sel_ps)
                    b128 = pact.tile([128, C], F32, name="b128", tag="b128", bufs=2)
                    c128 = pact.tile([128, C], F32, name="c128", tag="c128", bufs=2)
                    nc.sync.dma_start(out=b128[0:64, :], in_=bcn[0:64, :])
                    nc.sync.dma_start(out=b128[64:128, :], in_=bcn[0:64, :])
                    nc.sync.dma_start(out=c128[0:64, :], in_=bcn[64:128, :])
                    nc.sync.dma_start(out=c128[64:128, :], in_=bcn[64:128, :])

                # ======== SILU scalar block ========
                if cur is not None:
                    # conv gate silu (frees g psum)
                    gs = []
                    for m in range(2):
                        g = pact.tile([128, C], F32, name="gs", tag="gs", bufs=2)
                        nc.scalar.activation(out=g, in_=g_ps[m], func=AF.Silu)
                        gs.append(g)

                    # ---- DVE: depthwise conv FMA chain + gate
                    conv_out = []
                    for m in range(2):
                        cc = pact.tile([128, C], F32, name="cc", tag="cc", bufs=2)
                        nc.vector.tensor_scalar(
                            out=cc, in0=u_ext[m][:, 0:C], scalar1=convw[m][:, 0:1],
                            scalar2=convb[m], op0=OP.mult, op1=OP.add,
                        )
                        for kk in (1, 2):
                            nc.vector.scalar_tensor_tensor(
                                out=cc, in0=u_ext[m][:, kk : C + kk],
                                scalar=convw[m][:, kk : kk + 1], in1=cc,
                                op0=OP.mult, op1=OP.add,
                            )
                        co = pact.tile([128, C], BF16, name="convout", tag="convout", bufs=3)
                        nc.vector.tensor_mul(out=co, in0=cc, in1=gs[m])
                        conv_out.append(co)

                    # ---- scan chain (DVE + gpsimd)
                    yT = []
                    for m in range(2):
                        tmp = pact.tile([128, C], F32, name="tmp", tag="tmp", bufs=2)
                        nc.vector.tensor_mul(out=tmp, in0=dtf[m], in1=xssm_sb[m])
                        inp = pact.tile([128, C], F32, name="inp", tag="inp", bufs=2)
                        nc.gpsimd.tensor_mul(out=inp, in0=tmp, in1=b128)
                        hs = pact.tile([128, C], F32, name="hs", tag="hs", bufs=2)
                        nc.vector.tensor_tensor_scan(
                            out=hs, data0=decay[m], data1=inp, initial=h_st[m],
                            op0=OP.mult, op1=OP.add,
                        )
                        nc.vector.tensor_copy(out=h_st[m], in_=hs[:, C - 1 : C])
                        hc = pact.tile([128, C], F32, name="hc", tag="hc", bufs=2)
                        nc.gpsimd.tensor_mul(out=hc, in0=hs, in1=c128)
                        yt = pact.tile([128, C], BF16, name="yt", tag="yt", bufs=2)
                        nc.vector.scalar_tensor_tensor(
                            out=yt, in0=xssm_sb[m], scalar=dvec[m], in1=hc,
                            op0=OP.mult, op1=OP.add,
                        )
                        yT.append(yt)
                    cur["yT"] = yT

                # ---- FFN of chunk i-1 (fp8 DoubleRow): w1/w3 + silu + gate + w2 li01
                if prev is not None:
                    n8 = prev["n8"]
                    psf01 = [psF.tile([128, D_MODEL], F32, name="psf", tag="psF") for _ in range(2)]
                    h8 = pffn.tile([128, NKF, C], F8, name="h8", tag="h8", bufs=2)
                    for kf in range(NKF):
                        pa = psP.tile([128, C], F32, name="psPa", tag="psP")
                        for kp in (0, 2):
                            mm(pa, w1_sb[:, kp : kp + 2, kf * 128 : (kf + 1) * 128],
                               n8[:, kp : kp + 2, :], start=(kp == 0), stop=(kp == 2), pm=DR)
                        pb = psP.tile([128, C], F32, name="psPb", tag="psP")
                        for kp in (0, 2):
                            mm(pb, w3_sb[:, kp : kp + 2, kf * 128 : (kf + 1) * 128],
                               n8[:, kp : kp + 2, :], start=(kp == 0), stop=(kp == 2), pm=DR)
                        h_t = pffn.tile([128, C], F32, name="ht", tag="ht", bufs=2)
                        nc.scalar.activation(out=h_t, in_=pa, func=AF.Silu, scale=1.0 / SFF)
                        nc.vector.tensor_mul(out=h8[:, kf, :], in0=pb, in1=h_t)
                        if kf % 2 == 1:
                            for li in range(2):
                                mm(psf01[li], h8[:, kf - 1 : kf + 1, li * 128 : (li + 1) * 128],
                                   w2_sb[:, kf - 1 : kf + 1, :],
                                   start=(kf == 1), stop=(kf == NKF - 1), pm=DR)
                    # residual (undo the fp8 weight scaling) + DMA out subtiles 0,1
                    # (frees psF slots so pass li23 can allocate during ssm/mixer)
                    for li in range(2):
                        xt = prev["x_nat"][li]
                        nc.vector.scalar_tensor_tensor(
                            out=xt, in0=psf01[li], scalar=1.0 / (SFF * SFF), in1=xt,
                            op0=OP.mult, op1=OP.add,
                        )
                        nc.sync.dma_start(
                            out=xo_d[(i - 1) * C + li * 128 : (i - 1) * C + (li + 1) * 128, :],
                            in_=xt,
                        )
                    prev["h8"] = h8

                # ---- PE: ssm_out + mixer for chunk i (scan done by now)
                if cur is not None:
                    y2T = []
                    for m in range(2):
                        ps = psP.tile([128, C], F32, name="psPy2", tag="psP")
                        for k in range(2):
                            mm(ps, wssm_sb[k][:, m * 128 : (m + 1) * 128], cur["yT"][k],
                               start=(k == 0), stop=(k == 1))
                        y2 = pact.tile([128, C], BF16, name="y2", tag="y2", bufs=2)
                        nc.vector.tensor_copy(out=y2, in_=ps)
                        y2T.append(y2)

                    mix_lhsT = [conv_out[0], conv_out[1], y2T[0], y2T[1]]
                    for li in range(NSUB):
                        ps = psT.tile([128, D_MODEL], F32, name="psTm", tag="psT")
                        for k in range(4):
                            mm(ps, mix_lhsT[k][:, li * 128 : (li + 1) * 128], wop_sb[k],
                               start=(k == 0), stop=(k == 3))
                        # v_new = beta*v + mixer  (in-place into v tile)
                        vt = cur["v_nat"][li]
                        nc.vector.scalar_tensor_tensor(
                            out=vt, in0=vt, scalar=beta, in1=ps,
                            op0=OP.mult, op1=OP.add,
                        )
                        nc.sync.dma_start(
                            out=vo_d[i * C + li * 128 : i * C + (li + 1) * 128, :], in_=vt
                        )
                        # x2 = x + v_new  (in-place into x tile, gpsimd)
                        xt = cur["x_nat"][li]
                        nc.gpsimd.tensor_add(out=xt, in0=xt, in1=vt)

                # ---- FFN pass li23 + final residual + DMA out for chunk i-1
                if prev is not None:
                    h8 = prev["h8"]
                    psf23 = [psF.tile([128, D_MODEL], F32, name="psf", tag="psF") for _ in range(2)]
                    for kf in range(1, NKF, 2):
                        for li in range(2):
                            mm(psf23[li], h8[:, kf - 1 : kf + 1, (2 + li) * 128 : (3 + li) * 128],
                               w2_sb[:, kf - 1 : kf + 1, :],
                               start=(kf == 1), stop=(kf == NKF - 1), pm=DR)
                    for li in range(2, NSUB):
                        xt = prev["x_nat"][li]
                        nc.vector.scalar_tensor_tensor(
                            out=xt, in0=psf23[li - 2], scalar=1.0 / (SFF * SFF), in1=xt,
                            op0=OP.mult, op1=OP.add,
                        )
                        nc.sync.dma_start(
                            out=xo_d[(i - 1) * C + li * 128 : (i - 1) * C + (li + 1) * 128, :],
                            in_=xt,
                        )

                prev = cur

    if split:
        split_waits(nc)
    return nc


# ---------------------------------------------------------------- host glue
def prep_weights(inputs):
    """Host-side preprocessing: fold norm weights into matmul weights,
    precompute A = -exp(A_log), beta, and small constant matrices."""
    f = lambda a: np.asarray(a, dtype=np.float32)
    bf = lambda a: np.ascontiguousarray(np.asarray(a, dtype=np.float32).astype(ml_dtypes.bfloat16))
    SFF = 32.0  # keep in sync with kernel SFF

    def f8_3d(a, scale):
        """[K, N] -> [128, K//128, N] fp8e4m3 with scale folded in (TRN fp8e4
        matches ml_dtypes.float8_e4m3 for |x| <= 240)."""
        a = np.asarray(a, dtype=np.float32) * scale
        K, N = a.shape
        a = a.reshape(K // 128, 128, N).transpose(1, 0, 2)
        return np.ascontiguousarray(a.astype(ml_dtypes.float8_e4m3))
    pre_w = f(inputs["pre_norm_w"])[:, None]
    ffn_w = f(inputs["ffn_norm_w"])[:, None]
    A = -np.exp(f(inputs["A_log"]).reshape(-1))
    mask2 = np.zeros((128, 2), np.float32)
    mask2[0:64, 0] = 1.0
    mask2[64:128, 1] = 1.0
    sel2 = np.zeros((2, 128), np.float32)
    sel2[0, 0:64] = 1.0
    sel2[1, 64:128] = 1.0
    beta = float(1.0 / (1.0 + np.exp(-f(inputs["log_beta"]))))
    wbc = np.concatenate([pre_w * f(inputs["B_w"]), pre_w * f(inputs["C_w"])], axis=1)
    w = {
        "w_conv": bf(pre_w * f(inputs["conv_in_w"])),
        "w_xproj": bf(pre_w * f(inputs["x_proj_w"])),
        "w_dt": bf(pre_w * f(inputs["dt_w"])),
        "w_bc": bf(wbc),
        "w_ssmout": bf(f(inputs["ssm_out_w"])),
        "w_outproj": bf(f(inputs["out_proj_w"])),
        "w1": f8_3d(ffn_w * f(inputs["w1"]), SFF),
        "w3": f8_3d(ffn_w * f(inputs["w3"]), SFF),
        "w2": f8_3d(f(inputs["w2"]), SFF),
        "a_vec": A[:, None].copy(),
        "dtb_vec": f(inputs["dt_b"])[:, None].copy(),
        "d_vec": f(inputs["D"])[:, None].copy(),
        "convb_vec": f(inputs["conv_dw_b"])[:, None].copy(),
        "convw": np.ascontiguousarray(f(inputs["conv_dw_w"])),
        "mask2": mask2,
        "sel2": sel2,
        "ident": np.ascontiguousarray(np.eye(128, dtype=np.float32).astype(ml_dtypes.bfloat16)),
    }
    return w, beta


CHUNK = 512

_PROG_CACHE = {}


def kernel(**inputs):
    """Full-input entry point: shard batch over the 8 NeuronCores (one batch
    element per core; the scan state is per-(batch,channel) so this is
    embarrassingly parallel), run the Bass program SPMD, regather."""
    w, beta = prep_weights(inputs)
    x = np.asarray(inputs["x"], np.float32)
    v = np.asarray(inputs["velocity"], np.float32)
    n_cores, L, _ = x.shape
    key = (L, CHUNK, beta)
    if key not in _PROG_CACHE:
        _PROG_CACHE[key] = build_program(L, CHUNK, beta)
    nc = _PROG_CACHE[key]
    in_maps = []
    for b in range(n_cores):
        m = dict(w)
        m["x"] = np.ascontiguousarray(x[b])
        m["v"] = np.ascontiguousarray(v[b])
        in_maps.append(m)
    res = run_bass_kernel_spmd(nc, in_maps, core_ids=list(range(n_cores)))
    x_out = np.stack([res.results[b]["x_out"] for b in range(n_cores)])
    v_out = np.stack([res.results[b]["v_out"] for b in range(n_cores)])
    return (x_out, v_out)


# revision 18
# speedup vs baseline: 1.7088x; 1.0793x over previous
"""CoreHybridBlock Trainium2 kernel: builder + host glue (v2).

Per-core program (one batch element per core), chunked over tokens (C=512):
  natural layout = [token(part), feature(free)], transposed = [feature(part), token(free)]

  Pipeline is skewed one chunk for the FFN so the scalar engine needs only
  two activation-table switches per iteration (nlexp set <-> silu set):

  iteration i:
    nlexp block: rmsnorm1(i) rsqrt via exp(-ln/2); rmsnorm2(i-1);
                 dt softplus via ln(1+exp); decay exp; B/C rownorm via
                 exp(-relu(ln)/2)  [clip(norm,1) done in log space]
    silu block:  conv gate silu(i); ffn silu(i-1); rmsnorm1-square(i+1)
    PE: xnT(i) transposes, projections(i), nT(i-1) transposes,
        ffn(i-1) w1/w3 + w2(pass li01), ssm_out(i), mixer(i), w2(pass li23)
    DVE: copies/casts, conv FMA chain, scan, gate muls, residual stts
    GpSimd: scan input/output muls, x2 residual add (SBUF-only fp32)

  All matmul operands bf16 (fp32 accumulation in PSUM); residual stream,
  scan, and scalar chains stay fp32.
"""

import ml_dtypes
import numpy as np
import bass_rust
import concourse.bass as bass
import concourse.tile as tile
from concourse import mybir
from concourse.bass_utils import run_bass_kernel_spmd

F32 = mybir.dt.float32
BF16 = mybir.dt.bfloat16
F8 = mybir.dt.float8e4
DR = mybir.MatmulPerfMode.DoubleRow
SFF = 32.0  # fp8 ffn weight scale (h absmax ~2.9 -> 32*h ~ 92 < 240)
AF = mybir.ActivationFunctionType
OP = mybir.AluOpType

D_MODEL, D_CONV, D_MAMBA = 512, 256, 256
DSTATE, N_HEADS, KCONV, FFN = 64, 4, 3, 2048
EPS = 1e-6
NKF = FFN // 128  # 16


# ---------------------------------------------------------------- wait split
def split_waits(nc, max_w=1):
    """walrus in this container rejects >~1 sync wait per instruction on some
    instruction types (the Tile end-drain carries one wait per live
    semaphore).  Hoist excess waits onto same-engine NoOps placed before the
    offending instruction."""
    cnt = 0
    for f in nc.m.functions:
        for bb in f.blocks:
            new_list = []
            changed = False
            for inst in bb.instructions:
                si = inst.sync_info
                waits = list(si.on_wait) if si is not None and si.on_wait else []
                if len(waits) > max_w:
                    changed = True
                    extra = waits[max_w:]
                    si.on_wait = waits[:max_w]
                    for j in range(0, len(extra), max_w):
                        cnt += 1
                        nop = bass_rust.InstNoOp(
                            name=f"I-waitsplit-{cnt}", ins=[], outs=[]
                        )
                        nop.engine = inst.engine
                        nop.sync_info = bass_rust.SyncInfo(
                            on_wait=extra[j : j + max_w], on_update=[]
                        )
                        new_list.append(nop)
                new_list.append(inst)
            if changed:
                bb.instructions = new_list
    return cnt


# ---------------------------------------------------------------- program
def build_program(L, C, beta, split=True):
    """One-core program; SPMD over 8 cores with different x/v slices."""
    NCH = L // C
    NSUB = C // 128  # token-subtiles per chunk (natural layout)
    nc = bass.Bass()

    # ---- dram I/O
    x_d = nc.dram_tensor("x", [L, D_MODEL], F32, kind="ExternalInput")
    v_d = nc.dram_tensor("v", [L, D_MODEL], F32, kind="ExternalInput")
    wconv_d = nc.dram_tensor("w_conv", [D_MODEL, 2 * D_CONV], BF16, kind="ExternalInput")
    wxp_d = nc.dram_tensor("w_xproj", [D_MODEL, D_MAMBA], BF16, kind="ExternalInput")
    wdt_d = nc.dram_tensor("w_dt", [D_MODEL, D_MAMBA], BF16, kind="ExternalInput")
    wbc_d = nc.dram_tensor("w_bc", [D_MODEL, 2 * DSTATE], BF16, kind="ExternalInput")
    wssm_d = nc.dram_tensor("w_ssmout", [D_MAMBA, D_MAMBA], BF16, kind="ExternalInput")
    wop_d = nc.dram_tensor("w_outproj", [D_MODEL, D_MODEL], BF16, kind="ExternalInput")
    w1_d = nc.dram_tensor("w1", [128, 4, FFN], F8, kind="ExternalInput")
    w3_d = nc.dram_tensor("w3", [128, 4, FFN], F8, kind="ExternalInput")
    w2_d = nc.dram_tensor("w2", [128, NKF, D_MODEL], F8, kind="ExternalInput")
    avec_d = nc.dram_tensor("a_vec", [D_MAMBA, 1], F32, kind="ExternalInput")
    dtb_d = nc.dram_tensor("dtb_vec", [D_MAMBA, 1], F32, kind="ExternalInput")
    dvec_d = nc.dram_tensor("d_vec", [D_MAMBA, 1], F32, kind="ExternalInput")
    convb_d = nc.dram_tensor("convb_vec", [D_CONV, 1], F32, kind="ExternalInput")
    convw_d = nc.dram_tensor("convw", [D_CONV, KCONV], F32, kind="ExternalInput")
    maskbc_d = nc.dram_tensor("maskbc", [128, 128], BF16, kind="ExternalInput")
    ident_d = nc.dram_tensor("ident", [128, 128], BF16, kind="ExternalInput")

    xo_d = nc.dram_tensor("x_out", [L, D_MODEL], F32, kind="ExternalOutput")
    vo_d = nc.dram_tensor("v_out", [L, D_MODEL], F32, kind="ExternalOutput")

    with tile.TileContext(nc) as tc:
        with (
            tc.tile_pool(name="consts", bufs=1) as cp,
            tc.tile_pool(name="state", bufs=1) as sp,
            tc.tile_pool(name="io", bufs=2) as pio,
            tc.tile_pool(name="act", bufs=2) as pact,
            tc.tile_pool(name="ffn", bufs=2) as pffn,
            tc.tile_pool(name="psT", bufs=2, space="PSUM") as psT,
            tc.tile_pool(name="psP", bufs=4, space="PSUM") as psP,
            tc.tile_pool(name="psF", bufs=2, space="PSUM") as psF,
        ):
            def mm(out, lhsT, rhs, start, stop, pm=None):
                nc.tensor.matmul(out=out, lhsT=lhsT, rhs=rhs, start=start, stop=stop, perf_mode=pm)

            # ---------------- constants / weights resident in SBUF
            def load_const(name, dram_ap, shape, dt=F32):
                t = cp.tile(shape, dt, name=name, tag=name)
                nc.sync.dma_start(out=t, in_=dram_ap)
                return t

            wconv_sb = [
                load_const(f"wconv{k}", wconv_d[k * 128 : (k + 1) * 128, :], [128, 2 * D_CONV], BF16)
                for k in range(4)
            ]
            wxp_sb = [
                load_const(f"wxp{k}", wxp_d[k * 128 : (k + 1) * 128, :], [128, D_MAMBA], BF16)
                for k in range(4)
            ]
            wdt_sb = [
                load_const(f"wdt{k}", wdt_d[k * 128 : (k + 1) * 128, :], [128, D_MAMBA], BF16)
                for k in range(4)
            ]
            wbc_sb = [
                load_const(f"wbc{k}", wbc_d[k * 128 : (k + 1) * 128, :], [128, 2 * DSTATE], BF16)
                for k in range(4)
            ]
            wssm_sb = [
                load_const(f"wssm{k}", wssm_d[k * 128 : (k + 1) * 128, :], [128, D_MAMBA], BF16)
                for k in range(2)
            ]
            wop_sb = [
                load_const(f"wop{k}", wop_d[k * 128 : (k + 1) * 128, :], [128, D_MODEL], BF16)
                for k in range(4)
            ]
            w1_sb = load_const("w1_sb", w1_d[:, :, :], [128, 4, FFN], F8)
            w3_sb = load_const("w3_sb", w3_d[:, :, :], [128, 4, FFN], F8)
            w2_sb = load_const("w2_sb", w2_d[:, :, :], [128, NKF, D_MODEL], F8)
            avec = [
                load_const(f"avec{m}", avec_d[m * 128 : (m + 1) * 128, :], [128, 1])
                for m in range(2)
            ]
            dtb = [
                load_const(f"dtb{m}", dtb_d[m * 128 : (m + 1) * 128, :], [128, 1])
                for m in range(2)
            ]
            dvec = [
                load_const(f"dvec{m}", dvec_d[m * 128 : (m + 1) * 128, :], [128, 1])
                for m in range(2)
            ]
            convb = [
                load_const(f"convb{m}", convb_d[m * 128 : (m + 1) * 128, :], [128, 1])
                for m in range(2)
            ]
            convw = [
                load_const(f"convw{m}", convw_d[m * 128 : (m + 1) * 128, :], [128, KCONV])
                for m in range(2)
            ]
            maskbc = load_const("maskbc", maskbc_d[:, :], [128, 128], BF16)
            ident = load_const("ident", ident_d[:, :], [128, 128], BF16)

            eps_sb = cp.tile([128, 1], F32, name="eps_sb", tag="eps_sb")
            nc.vector.memset(eps_sb, EPS)
            one_sb = cp.tile([128, 1], F32, name="one_sb", tag="one_sb")
            nc.vector.memset(one_sb, 1.0)

            # ---------------- persistent cross-chunk state
            h_st = [sp.tile([128, 1], F32, name=f"hst{m}", tag=f"hst{m}") for m in range(2)]
            u_halo = [sp.tile([128, 2], F32, name=f"uhalo{m}", tag=f"uhalo{m}") for m in range(2)]
            for m in range(2):
                nc.vector.memset(h_st[m], 0.0)
                nc.vector.memset(u_halo[m], 0.0)

            # ---------------- per-iteration state carried across the skew
            prev = None  # dict with chunk i-1 leftovers

            def load_chunk(i):
                row0 = i * C
                x_nat, v_nat = [], []
                for s in range(NSUB):
                    xt = pio.tile([128, D_MODEL], F32, name="xnat", tag="xnat", bufs=9)
                    nc.sync.dma_start(
                        out=xt, in_=x_d[row0 + s * 128 : row0 + (s + 1) * 128, :]
                    )
                    x_nat.append(xt)
                    vt = pio.tile([128, D_MODEL], F32, name="vnat", tag="vnat", bufs=5)
                    nc.sync.dma_start(
                        out=vt, in_=v_d[row0 + s * 128 : row0 + (s + 1) * 128, :]
                    )
                    v_nat.append(vt)
                return x_nat, v_nat

            def rms_squares(src_tiles, tag):
                """scalar Square + accum -> per-token sum of squares [128,1]x NSUB"""
                ssqs = []
                for s in range(NSUB):
                    scr = pact.tile([128, D_MODEL], F32, name="sqscr", tag="sqscr", bufs=1)
                    ssq = pact.tile([128, 1], F32, name="ssq", tag=tag, bufs=2 * NSUB)
                    nc.scalar.activation(out=scr, in_=src_tiles[s], func=AF.Square, accum_out=ssq)
                    ssqs.append(ssq)
                return ssqs

            def rms_finish(ssqs, tag):
                """scalar: r = exp(-0.5*ln(ms/D + eps))  [nlexp set]"""
                rs = []
                for s in range(NSUB):
                    r = pact.tile([128, 1], F32, name="rr", tag=tag, bufs=2 * NSUB)
                    nc.scalar.activation(
                        out=r, in_=ssqs[s], func=AF.Ln, scale=1.0 / D_MODEL, bias=eps_sb
                    )
                    nc.scalar.activation(out=r, in_=r, func=AF.Exp, scale=-0.5)
                    rs.append(r)
                return rs

            def rms_apply(src_tiles, rs, tag):
                """DVE: xn = x * r -> bf16"""
                outs = []
                for s in range(NSUB):
                    xn = pact.tile([128, D_MODEL], BF16, name=tag, tag=tag, bufs=NSUB + 1)
                    nc.vector.tensor_scalar(
                        out=xn, in0=src_tiles[s], scalar1=rs[s], scalar2=None, op0=OP.mult
                    )
                    outs.append(xn)
                return outs

            def transpose_tiles(nat_tiles, dst_tag):
                """NSUB x [128,D_MODEL](bf16) natural -> 4 x [128,C](bf16) transposed."""
                outT = []
                for d in range(4):
                    ps = psT.tile([128, C], BF16, name="psTt", tag="psT")
                    for s in range(NSUB):
                        nc.tensor.transpose(
                            out=ps[:, s * 128 : (s + 1) * 128],
                            in_=nat_tiles[s][:, d * 128 : (d + 1) * 128],
                            identity=ident,
                        )
                    t = pact.tile([128, C], BF16, name=dst_tag, tag=dst_tag, bufs=5)
                    nc.vector.tensor_copy(out=t, in_=ps)
                    outT.append(t)
                return outT

            # ================================================ main loop (skewed)
            for i in range(NCH + 1):
                cur = None
                if i < NCH:
                    cur = {}
                    # ---- DMA in + rmsnorm1 (squares emitted in prev silu block
                    # for i>0; here for i==0)
                    x_nat, v_nat = load_chunk(i)
                    cur["x_nat"], cur["v_nat"] = x_nat, v_nat

                    # ======== NLEXP scalar block for iteration i ========
                    # (Square is in every table set: placing it here costs no
                    # table load, and it only depends on the x DMA -- so the
                    # xnT critical path never waits on chunk i-1's mixer.)
                    ssq1 = rms_squares(x_nat, "ssq1")
                    r1 = rms_finish(ssq1, "r1")
                    xn = rms_apply(x_nat, r1, "xn")
                    cur["xn"] = xn

                # rmsnorm2 of chunk i-1 (x2 lives in x_nat tiles of i-1)
                if prev is not None:
                    ssq2 = rms_squares(prev["x_nat"], "ssq2")
                    r2 = rms_finish(ssq2, "r2")
                    n2 = rms_apply(prev["x_nat"], r2, "n2")
                    prev["n2"] = n2

                if cur is not None:
                    # ---- PE: transposes + projections
                    xnT = transpose_tiles(cur["xn"], "xnT")
                    cur["xnT"] = xnT

                    # conv input proj u (m=0,1)  [gate g deferred to later]
                    u_ps = []
                    for m in range(2):
                        ps = psP.tile([128, C], F32, name="psPu", tag="psP")
                        for k in range(4):
                            mm(ps, wconv_sb[k][:, m * 128 : (m + 1) * 128], xnT[k],
                               start=(k == 0), stop=(k == 3))
                        u_ps.append(ps)
                    # x_ssm
                    xssm_ps = []
                    for m in range(2):
                        ps = psP.tile([128, C], F32, name="psPxs", tag="psP")
                        for k in range(4):
                            mm(ps, wxp_sb[k][:, m * 128 : (m + 1) * 128], xnT[k],
                               start=(k == 0), stop=(k == 3))
                        xssm_ps.append(ps)
                    # dt raw
                    dt_ps = []
                    for m in range(2):
                        ps = psP.tile([128, C], F32, name="psPdt", tag="psP")
                        for k in range(4):
                            mm(ps, wdt_sb[k][:, m * 128 : (m + 1) * 128], xnT[k],
                               start=(k == 0), stop=(k == 3))
                        dt_ps.append(ps)
                    # B/C merged [128 out rows: 0-63 B, 64-127 C]
                    bc_ps = psP.tile([128, C], F32, name="psPbc", tag="psP")
                    for k in range(4):
                        mm(bc_ps, wbc_sb[k], xnT[k], start=(k == 0), stop=(k == 3))
                    # conv gate g (m=0,1) -- late alloc (consumed in silu block)
                    g_ps = []
                    for m in range(2):
                        ps = psP.tile([128, C], F32, name="psPg", tag="psP")
                        for k in range(4):
                            mm(ps, wconv_sb[k][:, (2 + m) * 128 : (3 + m) * 128], xnT[k],
                               start=(k == 0), stop=(k == 3))
                        g_ps.append(ps)

                    # ---- DVE: conv u -> SBUF with halo
                    u_ext = []
                    for m in range(2):
                        ue = pact.tile([128, C + 2], F32, name="uext", tag="uext", bufs=2)
                        nc.vector.tensor_copy(out=ue[:, 2 : C + 2], in_=u_ps[m])
                        nc.vector.tensor_copy(out=ue[:, 0:2], in_=u_halo[m])
                        nc.vector.tensor_copy(out=u_halo[m], in_=ue[:, C : C + 2])
                        u_ext.append(ue)

                    # ---- scalar: xssm copies to SBUF (frees PSUM)
                    xssm_sb = []
                    for m in range(2):
                        xs = pact.tile([128, C], F32, name="xssm", tag="xssm", bufs=2)
                        nc.scalar.copy(out=xs, in_=xssm_ps[m])
                        xssm_sb.append(xs)

                    # ---- scalar: dt softplus in-place in PSUM [nlexp]
                    dtf = []
                    for m in range(2):
                        nc.scalar.activation(out=dt_ps[m], in_=dt_ps[m], func=AF.Exp, bias=dtb[m])
                        nc.scalar.activation(out=dt_ps[m], in_=dt_ps[m], func=AF.Ln, bias=one_sb)
                        df = pact.tile([128, C], F32, name="dtf", tag="dtf", bufs=2)
                        nc.vector.tensor_scalar(
                            out=df, in0=dt_ps[m], scalar1=1e-4, scalar2=0.1,
                            op0=OP.max, op1=OP.min,
                        )
                        dtf.append(df)
                    # decay = exp(A*dt)
                    decay = []
                    for m in range(2):
                        dc = pact.tile([128, C], F32, name="dec", tag="dec", bufs=2)
                        nc.scalar.activation(out=dc, in_=dtf[m], func=AF.Exp, scale=avec[m])
                        decay.append(dc)

                    # scalar part of B/C row norm (sq for partition-reduce; bm copy)
                    sqbc = pact.tile([128, C], BF16, name="sqbc", tag="sqbc", bufs=2)
                    nc.scalar.activation(out=sqbc, in_=bc_ps, func=AF.Square)
                    bm_sb = pact.tile([128, C], F32, name="bmsb", tag="bmsb", bufs=2)
                    nc.scalar.copy(out=bm_sb, in_=bc_ps)

                # ---- PE: nT transposes for chunk i-1 (needs n2 from nlexp above;
                # keeps PE busy while the scalar engine works through the B/C chain)
                if prev is not None:
                    n8 = pffn.tile([128, 4, C], F8, name="n8", tag="n8", bufs=2)
                    for d in range(4):
                        ps = psT.tile([128, C], BF16, name="psTt", tag="psT")
                        for s in range(NSUB):
                            nc.tensor.transpose(
                                out=ps[:, s * 128 : (s + 1) * 128],
                                in_=prev["n2"][s][:, d * 128 : (d + 1) * 128],
                                identity=ident,
                            )
                        nc.vector.tensor_copy(out=n8[:, d, :], in_=ps)
                    prev["n8"] = n8

                if cur is not None:
                    # ---- B/C row norm: r = exp(-0.5*relu(ln(s)))  (== min(1/sqrt(s),1))
                    # One block-diag matmul both reduces the squares over the 64
                    # states and broadcasts the per-token sums to all partitions.
                    sum_ps = psP.tile([128, C], F32, name="psPs2", tag="psP")
                    mm(sum_ps, maskbc, sqbc, start=True, stop=True)
                    nc.scalar.activation(out=sum_ps, in_=sum_ps, func=AF.Ln)
                    nc.scalar.activation(out=sum_ps, in_=sum_ps, func=AF.Relu)
                    r128 = pact.tile([128, C], F32, name="r128", tag="r128", bufs=2)
                    nc.scalar.activation(out=r128, in_=sum_ps, func=AF.Exp, scale=-0.5)
                    bcn = pact.tile([128, C], F32, name="bcn", tag="bcn", bufs=2)
                    nc.vector.tensor_mul(out=bcn, in0=bm_sb, in1=r128)
                    b128 = pact.tile([128, C], F32, name="b128", tag="b128", bufs=2)
                    c128 = pact.tile([128, C], F32, name="c128", tag="c128", bufs=2)
                    nc.sync.dma_start(out=b128[0:64, :], in_=bcn[0:64, :])
                    nc.sync.dma_start(out=b128[64:128, :], in_=bcn[0:64, :])
                    nc.sync.dma_start(out=c128[0:64, :], in_=bcn[64:128, :])
                    nc.sync.dma_start(out=c128[64:128, :], in_=bcn[64:128, :])

                # ======== SILU scalar block ========
                if cur is not None:
                    # conv gate silu (frees g psum)
                    gs = []
                    for m in range(2):
                        g = pact.tile([128, C], F32, name="gs", tag="gs", bufs=2)
                        nc.scalar.activation(out=g, in_=g_ps[m], func=AF.Silu)
                        gs.append(g)

                    # ---- DVE: depthwise conv FMA chain + gate
                    conv_out = []
                    for m in range(2):
                        cc = pact.tile([128, C], F32, name="cc", tag="cc", bufs=2)
                        nc.vector.tensor_scalar(
                            out=cc, in0=u_ext[m][:, 0:C], scalar1=convw[m][:, 0:1],
                            scalar2=convb[m], op0=OP.mult, op1=OP.add,
                        )
                        for kk in (1, 2):
                            nc.vector.scalar_tensor_tensor(
                                out=cc, in0=u_ext[m][:, kk : C + kk],
                                scalar=convw[m][:, kk : kk + 1], in1=cc,
                                op0=OP.mult, op1=OP.add,
                            )
                        co = pact.tile([128, C], BF16, name="convout", tag="convout", bufs=3)
                        nc.vector.tensor_mul(out=co, in0=cc, in1=gs[m])
                        conv_out.append(co)

                    # ---- scan chain (DVE + gpsimd)
                    yT = []
                    for m in range(2):
                        tmp = pact.tile([128, C], F32, name="tmp", tag="tmp", bufs=2)
                        nc.gpsimd.tensor_mul(out=tmp, in0=dtf[m], in1=xssm_sb[m])
                        inp = pact.tile([128, C], F32, name="inp", tag="inp", bufs=2)
                        nc.gpsimd.tensor_mul(out=inp, in0=tmp, in1=b128)
                        hs = pact.tile([128, C], F32, name="hs", tag="hs", bufs=2)
                        nc.vector.tensor_tensor_scan(
                            out=hs, data0=decay[m], data1=inp, initial=h_st[m],
                            op0=OP.mult, op1=OP.add,
                        )
                        nc.vector.tensor_copy(out=h_st[m], in_=hs[:, C - 1 : C])
                        hc = pact.tile([128, C], F32, name="hc", tag="hc", bufs=2)
                        nc.gpsimd.tensor_mul(out=hc, in0=hs, in1=c128)
                        yt = pact.tile([128, C], BF16, name="yt", tag="yt", bufs=2)
                        nc.vector.scalar_tensor_tensor(
                            out=yt, in0=xssm_sb[m], scalar=dvec[m], in1=hc,
                            op0=OP.mult, op1=OP.add,
                        )
                        yT.append(yt)
                    cur["yT"] = yT

                # ---- FFN of chunk i-1 (fp8 DoubleRow): w1/w3 + silu + gate + w2 li01
                if prev is not None:
                    n8 = prev["n8"]
                    psf01 = [psF.tile([128, D_MODEL], F32, name="psf", tag="psF") for _ in range(2)]
                    h8 = pffn.tile([128, NKF, C], F8, name="h8", tag="h8", bufs=2)
                    for kf in range(NKF):
                        pa = psP.tile([128, C], F32, name="psPa", tag="psP")
                        for kp in (0, 2):
                            mm(pa, w1_sb[:, kp : kp + 2, kf * 128 : (kf + 1) * 128],
                               n8[:, kp : kp + 2, :], start=(kp == 0), stop=(kp == 2), pm=DR)
                        pb = psP.tile([128, C], F32, name="psPb", tag="psP")
                        for kp in (0, 2):
                            mm(pb, w3_sb[:, kp : kp + 2, kf * 128 : (kf + 1) * 128],
                               n8[:, kp : kp + 2, :], start=(kp == 0), stop=(kp == 2), pm=DR)
                        h_t = pffn.tile([128, C], F32, name="ht", tag="ht", bufs=2)
                        nc.scalar.activation(out=h_t, in_=pa, func=AF.Silu, scale=1.0 / SFF)
                        nc.vector.tensor_mul(out=h8[:, kf, :], in0=pb, in1=h_t)
                        if kf % 2 == 1:
                            for li in range(2):
                                mm(psf01[li], h8[:, kf - 1 : kf + 1, li * 128 : (li + 1) * 128],
                                   w2_sb[:, kf - 1 : kf + 1, :],
                                   start=(kf == 1), stop=(kf == NKF - 1), pm=DR)
                    # residual (undo the fp8 weight scaling) + DMA out subtiles 0,1
                    # (frees psF slots so pass li23 can allocate during ssm/mixer)
                    for li in range(2):
                        xt = prev["x_nat"][li]
                        nc.vector.scalar_tensor_tensor(
                            out=xt, in0=psf01[li], scalar=1.0 / (SFF * SFF), in1=xt,
                            op0=OP.mult, op1=OP.add,
                        )
                        nc.sync.dma_start(
                            out=xo_d[(i - 1) * C + li * 128 : (i - 1) * C + (li + 1) * 128, :],
                            in_=xt,
                        )
                    prev["h8"] = h8

                # ---- PE: ssm_out + mixer for chunk i (scan done by now)
                if cur is not None:
                    y2T = []
                    for m in range(2):
                        ps = psP.tile([128, C], F32, name="psPy2", tag="psP")
                        for k in range(2):
                            mm(ps, wssm_sb[k][:, m * 128 : (m + 1) * 128], cur["yT"][k],
                               start=(k == 0), stop=(k == 1))
                        y2 = pact.tile([128, C], BF16, name="y2", tag="y2", bufs=2)
                        nc.vector.tensor_copy(out=y2, in_=ps)
                        y2T.append(y2)

                    mix_lhsT = [conv_out[0], conv_out[1], y2T[0], y2T[1]]
                    for li in range(NSUB):
                        ps = psT.tile([128, D_MODEL], F32, name="psTm", tag="psT")
                        for k in range(4):
                            mm(ps, mix_lhsT[k][:, li * 128 : (li + 1) * 128], wop_sb[k],
                               start=(k == 0), stop=(k == 3))
                        # v_new = beta*v + mixer  (in-place into v tile)
                        vt = cur["v_nat"][li]
                        nc.vector.scalar_tensor_tensor(
                            out=vt, in0=vt, scalar=beta, in1=ps,
                            op0=OP.mult, op1=OP.add,
                        )
                        nc.sync.dma_start(
                            out=vo_d[i * C + li * 128 : i * C + (li + 1) * 128, :], in_=vt
                        )
                        # x2 = x + v_new  (in-place into x tile, gpsimd)
                        xt = cur["x_nat"][li]
                        nc.gpsimd.tensor_add(out=xt, in0=xt, in1=vt)

                # ---- FFN pass li23 + final residual + DMA out for chunk i-1
                if prev is not None:
                    h8 = prev["h8"]
                    psf23 = [psF.tile([128, D_MODEL], F32, name="psf", tag="psF") for _ in range(2)]
                    for kf in range(1, NKF, 2):
                        for li in range(2):
                            mm(psf23[li], h8[:, kf - 1 : kf + 1, (2 + li) * 128 : (3 + li) * 128],
                               w2_sb[:, kf - 1 : kf + 1, :],
                               start=(kf == 1), stop=(kf == NKF - 1), pm=DR)
                    for li in range(2, NSUB):
                        xt = prev["x_nat"][li]
                        nc.vector.scalar_tensor_tensor(
                            out=xt, in0=psf23[li - 2], scalar=1.0 / (SFF * SFF), in1=xt,
                            op0=OP.mult, op1=OP.add,
                        )
                        nc.sync.dma_start(
                            out=xo_d[(i - 1) * C + li * 128 : (i - 1) * C + (li + 1) * 128, :],
                            in_=xt,
                        )

                prev = cur

    if split:
        split_waits(nc)
    return nc


# ---------------------------------------------------------------- host glue
def prep_weights(inputs):
    """Host-side preprocessing: fold norm weights into matmul weights,
    precompute A = -exp(A_log), beta, and small constant matrices."""
    f = lambda a: np.asarray(a, dtype=np.float32)
    bf = lambda a: np.ascontiguousarray(np.asarray(a, dtype=np.float32).astype(ml_dtypes.bfloat16))
    SFF = 32.0  # keep in sync with kernel SFF

    def f8_3d(a, scale):
        """[K, N] -> [128, K//128, N] fp8e4m3 with scale folded in (TRN fp8e4
        matches ml_dtypes.float8_e4m3 for |x| <= 240)."""
        a = np.asarray(a, dtype=np.float32) * scale
        K, N = a.shape
        a = a.reshape(K // 128, 128, N).transpose(1, 0, 2)
        return np.ascontiguousarray(a.astype(ml_dtypes.float8_e4m3))
    pre_w = f(inputs["pre_norm_w"])[:, None]
    ffn_w = f(inputs["ffn_norm_w"])[:, None]
    A = -np.exp(f(inputs["A_log"]).reshape(-1))
    maskbc = np.zeros((128, 128), np.float32)
    maskbc[0:64, 0:64] = 1.0
    maskbc[64:128, 64:128] = 1.0
    beta = float(1.0 / (1.0 + np.exp(-f(inputs["log_beta"]))))
    wbc = np.concatenate([pre_w * f(inputs["B_w"]), pre_w * f(inputs["C_w"])], axis=1)
    w = {
        "w_conv": bf(pre_w * f(inputs["conv_in_w"])),
        "w_xproj": bf(pre_w * f(inputs["x_proj_w"])),
        "w_dt": bf(pre_w * f(inputs["dt_w"])),
        "w_bc": bf(wbc),
        "w_ssmout": bf(f(inputs["ssm_out_w"])),
        "w_outproj": bf(f(inputs["out_proj_w"])),
        "w1": f8_3d(ffn_w * f(inputs["w1"]), SFF),
        "w3": f8_3d(ffn_w * f(inputs["w3"]), SFF),
        "w2": f8_3d(f(inputs["w2"]), SFF),
        "a_vec": A[:, None].copy(),
        "dtb_vec": f(inputs["dt_b"])[:, None].copy(),
        "d_vec": f(inputs["D"])[:, None].copy(),
        "convb_vec": f(inputs["conv_dw_b"])[:, None].copy(),
        "convw": np.ascontiguousarray(f(inputs["conv_dw_w"])),
        "maskbc": np.ascontiguousarray(maskbc.astype(ml_dtypes.bfloat16)),
        "ident": np.ascontiguousarray(np.eye(128, dtype=np.float32).astype(ml_dtypes.bfloat16)),
    }
    return w, beta


CHUNK = 512

_PROG_CACHE = {}


def kernel(**inputs):
    """Full-input entry point: shard batch over the 8 NeuronCores (one batch
    element per core; the scan state is per-(batch,channel) so this is
    embarrassingly parallel), run the Bass program SPMD, regather."""
    w, beta = prep_weights(inputs)
    x = np.asarray(inputs["x"], np.float32)
    v = np.asarray(inputs["velocity"], np.float32)
    n_cores, L, _ = x.shape
    key = (L, CHUNK, beta)
    if key not in _PROG_CACHE:
        _PROG_CACHE[key] = build_program(L, CHUNK, beta)
    nc = _PROG_CACHE[key]
    in_maps = []
    for b in range(n_cores):
        m = dict(w)
        m["x"] = np.ascontiguousarray(x[b])
        m["v"] = np.ascontiguousarray(v[b])
        in_maps.append(m)
    res = run_bass_kernel_spmd(nc, in_maps, core_ids=list(range(n_cores)))
    x_out = np.stack([res.results[b]["x_out"] for b in range(n_cores)])
    v_out = np.stack([res.results[b]["v_out"] for b in range(n_cores)])
    return (x_out, v_out)


# revision 19
# speedup vs baseline: 1.7457x; 1.0216x over previous
"""CoreHybridBlock Trainium2 kernel: builder + host glue (v2).

Per-core program (one batch element per core), chunked over tokens (C=512):
  natural layout = [token(part), feature(free)], transposed = [feature(part), token(free)]

  Pipeline is skewed one chunk for the FFN so the scalar engine needs only
  two activation-table switches per iteration (nlexp set <-> silu set):

  iteration i:
    nlexp block: rmsnorm1(i) rsqrt via exp(-ln/2); rmsnorm2(i-1);
                 dt softplus via ln(1+exp); decay exp; B/C rownorm via
                 exp(-relu(ln)/2)  [clip(norm,1) done in log space]
    silu block:  conv gate silu(i); ffn silu(i-1); rmsnorm1-square(i+1)
    PE: xnT(i) transposes, projections(i), nT(i-1) transposes,
        ffn(i-1) w1/w3 + w2(pass li01), ssm_out(i), mixer(i), w2(pass li23)
    DVE: copies/casts, conv FMA chain, scan, gate muls, residual stts
    GpSimd: scan input/output muls, x2 residual add (SBUF-only fp32)

  All matmul operands bf16 (fp32 accumulation in PSUM); residual stream,
  scan, and scalar chains stay fp32.
"""

import ml_dtypes
import numpy as np
import bass_rust
import concourse.bass as bass
import concourse.tile as tile
from concourse import mybir
from concourse.bass_utils import run_bass_kernel_spmd

F32 = mybir.dt.float32
BF16 = mybir.dt.bfloat16
F8 = mybir.dt.float8e4
DR = mybir.MatmulPerfMode.DoubleRow
SFF = 32.0  # fp8 ffn weight scale (h absmax ~2.9 -> 32*h ~ 92 < 240)
AF = mybir.ActivationFunctionType
OP = mybir.AluOpType

D_MODEL, D_CONV, D_MAMBA = 512, 256, 256
DSTATE, N_HEADS, KCONV, FFN = 64, 4, 3, 2048
EPS = 1e-6
NKF = FFN // 128  # 16


# ---------------------------------------------------------------- wait split
def split_waits(nc, max_w=1):
    """walrus in this container rejects >~1 sync wait per instruction on some
    instruction types (the Tile end-drain carries one wait per live
    semaphore).  Hoist excess waits onto same-engine NoOps placed before the
    offending instruction."""
    cnt = 0
    for f in nc.m.functions:
        for bb in f.blocks:
            new_list = []
            changed = False
            for inst in bb.instructions:
                si = inst.sync_info
                waits = list(si.on_wait) if si is not None and si.on_wait else []
                if len(waits) > max_w:
                    changed = True
                    extra = waits[max_w:]
                    si.on_wait = waits[:max_w]
                    for j in range(0, len(extra), max_w):
                        cnt += 1
                        nop = bass_rust.InstNoOp(
                            name=f"I-waitsplit-{cnt}", ins=[], outs=[]
                        )
                        nop.engine = inst.engine
                        nop.sync_info = bass_rust.SyncInfo(
                            on_wait=extra[j : j + max_w], on_update=[]
                        )
                        new_list.append(nop)
                new_list.append(inst)
            if changed:
                bb.instructions = new_list
    return cnt


# ---------------------------------------------------------------- program
def build_program(L, C, beta, split=True):
    """One-core program; SPMD over 8 cores with different x/v slices."""
    NCH = L // C
    NSUB = C // 128  # token-subtiles per chunk (natural layout)
    nc = bass.Bass()

    # ---- dram I/O
    x_d = nc.dram_tensor("x", [L, D_MODEL], F32, kind="ExternalInput")
    v_d = nc.dram_tensor("v", [L, D_MODEL], F32, kind="ExternalInput")
    wconv_d = nc.dram_tensor("w_conv", [D_MODEL, 2 * D_CONV], BF16, kind="ExternalInput")
    wxp_d = nc.dram_tensor("w_xproj", [D_MODEL, D_MAMBA], BF16, kind="ExternalInput")
    wdt_d = nc.dram_tensor("w_dt", [D_MODEL, D_MAMBA], BF16, kind="ExternalInput")
    wbc_d = nc.dram_tensor("w_bc", [D_MODEL, 2 * DSTATE], BF16, kind="ExternalInput")
    wssm_d = nc.dram_tensor("w_ssmout", [D_MAMBA, D_MAMBA], BF16, kind="ExternalInput")
    wop_d = nc.dram_tensor("w_outproj", [D_MODEL, D_MODEL], BF16, kind="ExternalInput")
    w1_d = nc.dram_tensor("w1", [128, 4, FFN], F8, kind="ExternalInput")
    w3_d = nc.dram_tensor("w3", [128, 4, FFN], F8, kind="ExternalInput")
    w2_d = nc.dram_tensor("w2", [128, NKF, D_MODEL], F8, kind="ExternalInput")
    avec_d = nc.dram_tensor("a_vec", [D_MAMBA, 1], F32, kind="ExternalInput")
    dtb_d = nc.dram_tensor("dtb_vec", [D_MAMBA, 1], F32, kind="ExternalInput")
    dvec_d = nc.dram_tensor("d_vec", [D_MAMBA, 1], F32, kind="ExternalInput")
    convb_d = nc.dram_tensor("convb_vec", [D_CONV, 1], F32, kind="ExternalInput")
    convw_d = nc.dram_tensor("convw", [D_CONV, KCONV], F32, kind="ExternalInput")
    maskbc_d = nc.dram_tensor("maskbc", [128, 128], BF16, kind="ExternalInput")
    ident_d = nc.dram_tensor("ident", [128, 128], BF16, kind="ExternalInput")

    xo_d = nc.dram_tensor("x_out", [L, D_MODEL], F32, kind="ExternalOutput")
    vo_d = nc.dram_tensor("v_out", [L, D_MODEL], F32, kind="ExternalOutput")

    with tile.TileContext(nc) as tc:
        with (
            tc.tile_pool(name="consts", bufs=1) as cp,
            tc.tile_pool(name="state", bufs=1) as sp,
            tc.tile_pool(name="io", bufs=2) as pio,
            tc.tile_pool(name="act", bufs=2) as pact,
            tc.tile_pool(name="ffn", bufs=2) as pffn,
            tc.tile_pool(name="psT", bufs=2, space="PSUM") as psT,
            tc.tile_pool(name="psP", bufs=4, space="PSUM") as psP,
            tc.tile_pool(name="psF", bufs=2, space="PSUM") as psF,
        ):
            def mm(out, lhsT, rhs, start, stop, pm=None):
                nc.tensor.matmul(out=out, lhsT=lhsT, rhs=rhs, start=start, stop=stop, perf_mode=pm)

            # ---------------- constants / weights resident in SBUF
            def load_const(name, dram_ap, shape, dt=F32):
                t = cp.tile(shape, dt, name=name, tag=name)
                nc.sync.dma_start(out=t, in_=dram_ap)
                return t

            wconv_sb = [
                load_const(f"wconv{k}", wconv_d[k * 128 : (k + 1) * 128, :], [128, 2 * D_CONV], BF16)
                for k in range(4)
            ]
            wxp_sb = [
                load_const(f"wxp{k}", wxp_d[k * 128 : (k + 1) * 128, :], [128, D_MAMBA], BF16)
                for k in range(4)
            ]
            wdt_sb = [
                load_const(f"wdt{k}", wdt_d[k * 128 : (k + 1) * 128, :], [128, D_MAMBA], BF16)
                for k in range(4)
            ]
            wbc_sb = [
                load_const(f"wbc{k}", wbc_d[k * 128 : (k + 1) * 128, :], [128, 2 * DSTATE], BF16)
                for k in range(4)
            ]
            wssm_sb = [
                load_const(f"wssm{k}", wssm_d[k * 128 : (k + 1) * 128, :], [128, D_MAMBA], BF16)
                for k in range(2)
            ]
            wop_sb = [
                load_const(f"wop{k}", wop_d[k * 128 : (k + 1) * 128, :], [128, D_MODEL], BF16)
                for k in range(4)
            ]
            w1_sb = load_const("w1_sb", w1_d[:, :, :], [128, 4, FFN], F8)
            w3_sb = load_const("w3_sb", w3_d[:, :, :], [128, 4, FFN], F8)
            w2_sb = load_const("w2_sb", w2_d[:, :, :], [128, NKF, D_MODEL], F8)
            avec = [
                load_const(f"avec{m}", avec_d[m * 128 : (m + 1) * 128, :], [128, 1])
                for m in range(2)
            ]
            dtb = [
                load_const(f"dtb{m}", dtb_d[m * 128 : (m + 1) * 128, :], [128, 1])
                for m in range(2)
            ]
            dvec = [
                load_const(f"dvec{m}", dvec_d[m * 128 : (m + 1) * 128, :], [128, 1])
                for m in range(2)
            ]
            convb = [
                load_const(f"convb{m}", convb_d[m * 128 : (m + 1) * 128, :], [128, 1])
                for m in range(2)
            ]
            convw = [
                load_const(f"convw{m}", convw_d[m * 128 : (m + 1) * 128, :], [128, KCONV])
                for m in range(2)
            ]
            maskbc = load_const("maskbc", maskbc_d[:, :], [128, 128], BF16)
            ident = load_const("ident", ident_d[:, :], [128, 128], BF16)

            eps_sb = cp.tile([128, 1], F32, name="eps_sb", tag="eps_sb")
            nc.vector.memset(eps_sb, EPS)
            one_sb = cp.tile([128, 1], F32, name="one_sb", tag="one_sb")
            nc.vector.memset(one_sb, 1.0)

            # ---------------- persistent cross-chunk state
            h_st = [sp.tile([128, 1], F32, name=f"hst{m}", tag=f"hst{m}") for m in range(2)]
            u_halo = [sp.tile([128, 2], F32, name=f"uhalo{m}", tag=f"uhalo{m}") for m in range(2)]
            for m in range(2):
                nc.vector.memset(h_st[m], 0.0)
                nc.vector.memset(u_halo[m], 0.0)

            # ---------------- per-iteration state carried across the skew
            prev = None  # dict with chunk i-1 leftovers
            ffn_q = []  # chunks whose FFN is pending (paired: 2 per flush)

            def load_chunk(i):
                row0 = i * C
                x_nat, v_nat = [], []
                for s in range(NSUB):
                    xt = pio.tile([128, D_MODEL], F32, name="xnat", tag="xnat", bufs=13)
                    nc.sync.dma_start(
                        out=xt, in_=x_d[row0 + s * 128 : row0 + (s + 1) * 128, :]
                    )
                    x_nat.append(xt)
                    vt = pio.tile([128, D_MODEL], F32, name="vnat", tag="vnat", bufs=5)
                    nc.sync.dma_start(
                        out=vt, in_=v_d[row0 + s * 128 : row0 + (s + 1) * 128, :]
                    )
                    v_nat.append(vt)
                return x_nat, v_nat

            def rms_squares(src_tiles, tag):
                """scalar Square + accum -> per-token sum of squares [128,1]x NSUB"""
                ssqs = []
                for s in range(NSUB):
                    scr = pact.tile([128, D_MODEL], F32, name="sqscr", tag="sqscr", bufs=1)
                    ssq = pact.tile([128, 1], F32, name="ssq", tag=tag, bufs=2 * NSUB)
                    nc.scalar.activation(out=scr, in_=src_tiles[s], func=AF.Square, accum_out=ssq)
                    ssqs.append(ssq)
                return ssqs

            def rms_finish(ssqs, tag):
                """scalar: r = exp(-0.5*ln(ms/D + eps))  [nlexp set]"""
                rs = []
                for s in range(NSUB):
                    r = pact.tile([128, 1], F32, name="rr", tag=tag, bufs=2 * NSUB)
                    nc.scalar.activation(
                        out=r, in_=ssqs[s], func=AF.Ln, scale=1.0 / D_MODEL, bias=eps_sb
                    )
                    nc.scalar.activation(out=r, in_=r, func=AF.Exp, scale=-0.5)
                    rs.append(r)
                return rs

            def rms_apply(src_tiles, rs, tag):
                """DVE: xn = x * r -> bf16"""
                outs = []
                for s in range(NSUB):
                    xn = pact.tile([128, D_MODEL], BF16, name=tag, tag=tag, bufs=NSUB + 1)
                    nc.vector.tensor_scalar(
                        out=xn, in0=src_tiles[s], scalar1=rs[s], scalar2=None, op0=OP.mult
                    )
                    outs.append(xn)
                return outs

            def transpose_tiles(nat_tiles, dst_tag):
                """NSUB x [128,D_MODEL](bf16) natural -> 4 x [128,C](bf16) transposed."""
                outT = []
                for d in range(4):
                    ps = psT.tile([128, C], BF16, name="psTt", tag="psT")
                    for s in range(NSUB):
                        nc.tensor.transpose(
                            out=ps[:, s * 128 : (s + 1) * 128],
                            in_=nat_tiles[s][:, d * 128 : (d + 1) * 128],
                            identity=ident,
                        )
                    t = pact.tile([128, C], BF16, name=dst_tag, tag=dst_tag, bufs=5)
                    nc.vector.tensor_copy(out=t, in_=ps)
                    outT.append(t)
                return outT

            # ================================================ main loop (skewed)
            for i in range(NCH + 1):
                cur = None
                if i < NCH:
                    cur = {"idx": i}
                    # ---- DMA in + rmsnorm1 (squares emitted in prev silu block
                    # for i>0; here for i==0)
                    x_nat, v_nat = load_chunk(i)
                    cur["x_nat"], cur["v_nat"] = x_nat, v_nat

                    # ======== NLEXP scalar block for iteration i ========
                    # (Square is in every table set: placing it here costs no
                    # table load, and it only depends on the x DMA -- so the
                    # xnT critical path never waits on chunk i-1's mixer.)
                    ssq1 = rms_squares(x_nat, "ssq1")
                    r1 = rms_finish(ssq1, "r1")
                    xn = rms_apply(x_nat, r1, "xn")
                    cur["xn"] = xn

                # rmsnorm2 of chunk i-1 (x2 lives in x_nat tiles of i-1)
                if prev is not None:
                    ssq2 = rms_squares(prev["x_nat"], "ssq2")
                    r2 = rms_finish(ssq2, "r2")
                    n2 = rms_apply(prev["x_nat"], r2, "n2")
                    prev["n2"] = n2

                if cur is not None:
                    # ---- PE: transposes + projections
                    xnT = transpose_tiles(cur["xn"], "xnT")
                    cur["xnT"] = xnT

                    # conv input proj u (m=0,1)  [gate g deferred to later]
                    u_ps = []
                    for m in range(2):
                        ps = psP.tile([128, C], F32, name="psPu", tag="psP")
                        for k in range(4):
                            mm(ps, wconv_sb[k][:, m * 128 : (m + 1) * 128], xnT[k],
                               start=(k == 0), stop=(k == 3))
                        u_ps.append(ps)
                    # x_ssm
                    xssm_ps = []
                    for m in range(2):
                        ps = psP.tile([128, C], F32, name="psPxs", tag="psP")
                        for k in range(4):
                            mm(ps, wxp_sb[k][:, m * 128 : (m + 1) * 128], xnT[k],
                               start=(k == 0), stop=(k == 3))
                        xssm_ps.append(ps)
                    # dt raw
                    dt_ps = []
                    for m in range(2):
                        ps = psP.tile([128, C], F32, name="psPdt", tag="psP")
                        for k in range(4):
                            mm(ps, wdt_sb[k][:, m * 128 : (m + 1) * 128], xnT[k],
                               start=(k == 0), stop=(k == 3))
                        dt_ps.append(ps)
                    # B/C merged [128 out rows: 0-63 B, 64-127 C]
                    bc_ps = psP.tile([128, C], F32, name="psPbc", tag="psP")
                    for k in range(4):
                        mm(bc_ps, wbc_sb[k], xnT[k], start=(k == 0), stop=(k == 3))
                    # conv gate g (m=0,1) -- late alloc (consumed in silu block)
                    g_ps = []
                    for m in range(2):
                        ps = psP.tile([128, C], F32, name="psPg", tag="psP")
                        for k in range(4):
                            mm(ps, wconv_sb[k][:, (2 + m) * 128 : (3 + m) * 128], xnT[k],
                               start=(k == 0), stop=(k == 3))
                        g_ps.append(ps)

                    # ---- DVE: conv u -> SBUF with halo
                    u_ext = []
                    for m in range(2):
                        ue = pact.tile([128, C + 2], F32, name="uext", tag="uext", bufs=2)
                        nc.vector.tensor_copy(out=ue[:, 2 : C + 2], in_=u_ps[m])
                        nc.vector.tensor_copy(out=ue[:, 0:2], in_=u_halo[m])
                        nc.vector.tensor_copy(out=u_halo[m], in_=ue[:, C : C + 2])
                        u_ext.append(ue)

                    # ---- scalar: xssm copies to SBUF (frees PSUM)
                    xssm_sb = []
                    for m in range(2):
                        xs = pact.tile([128, C], F32, name="xssm", tag="xssm", bufs=2)
                        nc.scalar.copy(out=xs, in_=xssm_ps[m])
                        xssm_sb.append(xs)

                    # ---- scalar: dt softplus in-place in PSUM [nlexp]
                    dtf = []
                    for m in range(2):
                        nc.scalar.activation(out=dt_ps[m], in_=dt_ps[m], func=AF.Exp, bias=dtb[m])
                        nc.scalar.activation(out=dt_ps[m], in_=dt_ps[m], func=AF.Ln, bias=one_sb)
                        df = pact.tile([128, C], F32, name="dtf", tag="dtf", bufs=2)
                        nc.vector.tensor_scalar(
                            out=df, in0=dt_ps[m], scalar1=1e-4, scalar2=0.1,
                            op0=OP.max, op1=OP.min,
                        )
                        dtf.append(df)
                    # decay = exp(A*dt)
                    decay = []
                    for m in range(2):
                        dc = pact.tile([128, C], F32, name="dec", tag="dec", bufs=2)
                        nc.scalar.activation(out=dc, in_=dtf[m], func=AF.Exp, scale=avec[m])
                        decay.append(dc)

                    # scalar part of B/C row norm (sq for partition-reduce; bm copy)
                    sqbc = pact.tile([128, C], BF16, name="sqbc", tag="sqbc", bufs=2)
                    nc.scalar.activation(out=sqbc, in_=bc_ps, func=AF.Square)
                    bm_sb = pact.tile([128, C], F32, name="bmsb", tag="bmsb", bufs=2)
                    nc.scalar.copy(out=bm_sb, in_=bc_ps)

                # ---- PE: nT transposes for chunk i-1 (needs n2 from nlexp above;
                # keeps PE busy while the scalar engine works through the B/C chain)
                if prev is not None:
                    n8 = pffn.tile([128, 4, C], F8, name="n8", tag="n8", bufs=2)
                    for d in range(4):
                        ps = psT.tile([128, C], BF16, name="psTt", tag="psT")
                        for s in range(NSUB):
                            nc.tensor.transpose(
                                out=ps[:, s * 128 : (s + 1) * 128],
                                in_=prev["n2"][s][:, d * 128 : (d + 1) * 128],
                                identity=ident,
                            )
                        nc.vector.tensor_copy(out=n8[:, d, :], in_=ps)
                    prev["n8"] = n8
                    ffn_q.append(prev)

                if cur is not None:
                    # ---- B/C row norm: r = exp(-0.5*relu(ln(s)))  (== min(1/sqrt(s),1))
                    # One block-diag matmul both reduces the squares over the 64
                    # states and broadcasts the per-token sums to all partitions.
                    sum_ps = psP.tile([128, C], F32, name="psPs2", tag="psP")
                    mm(sum_ps, maskbc, sqbc, start=True, stop=True)
                    nc.scalar.activation(out=sum_ps, in_=sum_ps, func=AF.Ln)
                    nc.scalar.activation(out=sum_ps, in_=sum_ps, func=AF.Relu)
                    r128 = pact.tile([128, C], F32, name="r128", tag="r128", bufs=2)
                    nc.scalar.activation(out=r128, in_=sum_ps, func=AF.Exp, scale=-0.5)
                    bcn = pact.tile([128, C], F32, name="bcn", tag="bcn", bufs=2)
                    nc.vector.tensor_mul(out=bcn, in0=bm_sb, in1=r128)
                    b128 = pact.tile([128, C], F32, name="b128", tag="b128", bufs=2)
                    c128 = pact.tile([128, C], F32, name="c128", tag="c128", bufs=2)
                    nc.sync.dma_start(out=b128[0:64, :], in_=bcn[0:64, :])
                    nc.sync.dma_start(out=b128[64:128, :], in_=bcn[0:64, :])
                    nc.sync.dma_start(out=c128[0:64, :], in_=bcn[64:128, :])
                    nc.sync.dma_start(out=c128[64:128, :], in_=bcn[64:128, :])

                # ======== SILU scalar block ========
                if cur is not None:
                    # conv gate silu (frees g psum)
                    gs = []
                    for m in range(2):
                        g = pact.tile([128, C], F32, name="gs", tag="gs", bufs=2)
                        nc.scalar.activation(out=g, in_=g_ps[m], func=AF.Silu)
                        gs.append(g)

                    # ---- DVE: depthwise conv FMA chain + gate
                    conv_out = []
                    for m in range(2):
                        cc = pact.tile([128, C], F32, name="cc", tag="cc", bufs=2)
                        nc.vector.tensor_scalar(
                            out=cc, in0=u_ext[m][:, 0:C], scalar1=convw[m][:, 0:1],
                            scalar2=convb[m], op0=OP.mult, op1=OP.add,
                        )
                        for kk in (1, 2):
                            nc.vector.scalar_tensor_tensor(
                                out=cc, in0=u_ext[m][:, kk : C + kk],
                                scalar=convw[m][:, kk : kk + 1], in1=cc,
                                op0=OP.mult, op1=OP.add,
                            )
                        co = pact.tile([128, C], BF16, name="convout", tag="convout", bufs=3)
                        nc.vector.tensor_mul(out=co, in0=cc, in1=gs[m])
                        conv_out.append(co)

                    # ---- scan chain (DVE + gpsimd)
                    yT = []
                    for m in range(2):
                        tmp = pact.tile([128, C], F32, name="tmp", tag="tmp", bufs=2)
                        nc.gpsimd.tensor_mul(out=tmp, in0=dtf[m], in1=xssm_sb[m])
                        inp = pact.tile([128, C], F32, name="inp", tag="inp", bufs=2)
                        nc.gpsimd.tensor_mul(out=inp, in0=tmp, in1=b128)
                        hs = pact.tile([128, C], F32, name="hs", tag="hs", bufs=2)
                        nc.vector.tensor_tensor_scan(
                            out=hs, data0=decay[m], data1=inp, initial=h_st[m],
                            op0=OP.mult, op1=OP.add,
                        )
                        nc.vector.tensor_copy(out=h_st[m], in_=hs[:, C - 1 : C])
                        hc = pact.tile([128, C], F32, name="hc", tag="hc", bufs=2)
                        nc.gpsimd.tensor_mul(out=hc, in0=hs, in1=c128)
                        yt = pact.tile([128, C], BF16, name="yt", tag="yt", bufs=2)
                        nc.vector.scalar_tensor_tensor(
                            out=yt, in0=xssm_sb[m], scalar=dvec[m], in1=hc,
                            op0=OP.mult, op1=OP.add,
                        )
                        yT.append(yt)
                    cur["yT"] = yT

                # ---- paired FFN (fp8 DoubleRow) for the two queued chunks.
                # Emitting both chunks' matmuls per weight slice back-to-back lets
                # the PE reuse the stationary operand (walrus skips the reload).
                pair = None
                if len(ffn_q) == 2:
                    pair = ffn_q
                    ffn_q = []
                    for q in pair:
                        q["h8"] = pffn.tile([128, NKF, C], F8, name="h8", tag="h8", bufs=2)
                    qA, qB = pair
                    psf01A = [psF.tile([128, D_MODEL], F32, name="psf", tag="psF") for _ in range(2)]
                    for kf in range(NKF):
                        pas, pbs = [], []
                        for q in pair:
                            pas.append(psP.tile([128, C], F32, name="psPa", tag="psP"))
                        for kp in (0, 2):
                            for q, pa in zip(pair, pas):
                                mm(pa, w1_sb[:, kp : kp + 2, kf * 128 : (kf + 1) * 128],
                                   q["n8"][:, kp : kp + 2, :],
                                   start=(kp == 0), stop=(kp == 2), pm=DR)
                        for q in pair:
                            pbs.append(psP.tile([128, C], F32, name="psPb", tag="psP"))
                        for kp in (0, 2):
                            for q, pb in zip(pair, pbs):
                                mm(pb, w3_sb[:, kp : kp + 2, kf * 128 : (kf + 1) * 128],
                                   q["n8"][:, kp : kp + 2, :],
                                   start=(kp == 0), stop=(kp == 2), pm=DR)
                        for q, pa, pb in zip(pair, pas, pbs):
                            h_t = pffn.tile([128, C], F32, name="ht", tag="ht", bufs=2)
                            nc.scalar.activation(out=h_t, in_=pa, func=AF.Silu, scale=1.0 / SFF)
                            nc.vector.tensor_mul(out=q["h8"][:, kf, :], in0=pb, in1=h_t)
                        if kf % 2 == 1:
                            for li in range(2):
                                mm(psf01A[li],
                                   qA["h8"][:, kf - 1 : kf + 1, li * 128 : (li + 1) * 128],
                                   w2_sb[:, kf - 1 : kf + 1, :],
                                   start=(kf == 1), stop=(kf == NKF - 1), pm=DR)
                    # chunk A: residual li01 (undo fp8 scale), then li23
                    iA = qA["idx"]
                    for li in range(2):
                        xt = qA["x_nat"][li]
                        nc.vector.scalar_tensor_tensor(
                            out=xt, in0=psf01A[li], scalar=1.0 / (SFF * SFF), in1=xt,
                            op0=OP.mult, op1=OP.add,
                        )
                        nc.sync.dma_start(
                            out=xo_d[iA * C + li * 128 : iA * C + (li + 1) * 128, :], in_=xt
                        )
                    psf23A = [psF.tile([128, D_MODEL], F32, name="psf", tag="psF") for _ in range(2)]
                    for kf in range(1, NKF, 2):
                        for li in range(2):
                            mm(psf23A[li],
                               qA["h8"][:, kf - 1 : kf + 1, (2 + li) * 128 : (3 + li) * 128],
                               w2_sb[:, kf - 1 : kf + 1, :],
                               start=(kf == 1), stop=(kf == NKF - 1), pm=DR)
                    for li in range(2, NSUB):
                        xt = qA["x_nat"][li]
                        nc.vector.scalar_tensor_tensor(
                            out=xt, in0=psf23A[li - 2], scalar=1.0 / (SFF * SFF), in1=xt,
                            op0=OP.mult, op1=OP.add,
                        )
                        nc.sync.dma_start(
                            out=xo_d[iA * C + li * 128 : iA * C + (li + 1) * 128, :], in_=xt
                        )

                # ---- PE: ssm_out + mixer for chunk i (scan done by now)
                if cur is not None:
                    y2T = []
                    for m in range(2):
                        ps = psP.tile([128, C], F32, name="psPy2", tag="psP")
                        for k in range(2):
                            mm(ps, wssm_sb[k][:, m * 128 : (m + 1) * 128], cur["yT"][k],
                               start=(k == 0), stop=(k == 1))
                        y2 = pact.tile([128, C], BF16, name="y2", tag="y2", bufs=2)
                        nc.vector.tensor_copy(out=y2, in_=ps)
                        y2T.append(y2)

                    mix_lhsT = [conv_out[0], conv_out[1], y2T[0], y2T[1]]
                    for li in range(NSUB):
                        ps = psT.tile([128, D_MODEL], F32, name="psTm", tag="psT")
                        for k in range(4):
                            mm(ps, mix_lhsT[k][:, li * 128 : (li + 1) * 128], wop_sb[k],
                               start=(k == 0), stop=(k == 3))
                        # v_new = beta*v + mixer  (in-place into v tile)
                        vt = cur["v_nat"][li]
                        nc.vector.scalar_tensor_tensor(
                            out=vt, in0=vt, scalar=beta, in1=ps,
                            op0=OP.mult, op1=OP.add,
                        )
                        nc.sync.dma_start(
                            out=vo_d[i * C + li * 128 : i * C + (li + 1) * 128, :], in_=vt
                        )
                        # x2 = x + v_new  (in-place into x tile, gpsimd)
                        xt = cur["x_nat"][li]
                        nc.gpsimd.tensor_add(out=xt, in0=xt, in1=vt)

                # ---- chunk B of the pair: w2 for all token subtiles + residual
                if pair is not None:
                    qB = pair[1]
                    iB = qB["idx"]
                    for half in range(2):
                        psfB = [psF.tile([128, D_MODEL], F32, name="psf", tag="psF") for _ in range(2)]
                        for kf in range(1, NKF, 2):
                            for li in range(2):
                                mm(psfB[li],
                                   qB["h8"][:, kf - 1 : kf + 1, (2 * half + li) * 128 : (2 * half + li + 1) * 128],
                                   w2_sb[:, kf - 1 : kf + 1, :],
                                   start=(kf == 1), stop=(kf == NKF - 1), pm=DR)
                        for li in range(2):
                            sub = 2 * half + li
                            xt = qB["x_nat"][sub]
                            nc.vector.scalar_tensor_tensor(
                                out=xt, in0=psfB[li], scalar=1.0 / (SFF * SFF), in1=xt,
                                op0=OP.mult, op1=OP.add,
                            )
                            nc.sync.dma_start(
                                out=xo_d[iB * C + sub * 128 : iB * C + (sub + 1) * 128, :],
                                in_=xt,
                            )

                prev = cur

    if split:
        split_waits(nc)
    return nc


# ---------------------------------------------------------------- host glue
def prep_weights(inputs):
    """Host-side preprocessing: fold norm weights into matmul weights,
    precompute A = -exp(A_log), beta, and small constant matrices."""
    f = lambda a: np.asarray(a, dtype=np.float32)
    bf = lambda a: np.ascontiguousarray(np.asarray(a, dtype=np.float32).astype(ml_dtypes.bfloat16))
    SFF = 32.0  # keep in sync with kernel SFF

    def f8_3d(a, scale):
        """[K, N] -> [128, K//128, N] fp8e4m3 with scale folded in (TRN fp8e4
        matches ml_dtypes.float8_e4m3 for |x| <= 240)."""
        a = np.asarray(a, dtype=np.float32) * scale
        K, N = a.shape
        a = a.reshape(K // 128, 128, N).transpose(1, 0, 2)
        return np.ascontiguousarray(a.astype(ml_dtypes.float8_e4m3))
    pre_w = f(inputs["pre_norm_w"])[:, None]
    ffn_w = f(inputs["ffn_norm_w"])[:, None]
    A = -np.exp(f(inputs["A_log"]).reshape(-1))
    maskbc = np.zeros((128, 128), np.float32)
    maskbc[0:64, 0:64] = 1.0
    maskbc[64:128, 64:128] = 1.0
    beta = float(1.0 / (1.0 + np.exp(-f(inputs["log_beta"]))))
    wbc = np.concatenate([pre_w * f(inputs["B_w"]), pre_w * f(inputs["C_w"])], axis=1)
    w = {
        "w_conv": bf(pre_w * f(inputs["conv_in_w"])),
        "w_xproj": bf(pre_w * f(inputs["x_proj_w"])),
        "w_dt": bf(pre_w * f(inputs["dt_w"])),
        "w_bc": bf(wbc),
        "w_ssmout": bf(f(inputs["ssm_out_w"])),
        "w_outproj": bf(f(inputs["out_proj_w"])),
        "w1": f8_3d(ffn_w * f(inputs["w1"]), SFF),
        "w3": f8_3d(ffn_w * f(inputs["w3"]), SFF),
        "w2": f8_3d(f(inputs["w2"]), SFF),
        "a_vec": A[:, None].copy(),
        "dtb_vec": f(inputs["dt_b"])[:, None].copy(),
        "d_vec": f(inputs["D"])[:, None].copy(),
        "convb_vec": f(inputs["conv_dw_b"])[:, None].copy(),
        "convw": np.ascontiguousarray(f(inputs["conv_dw_w"])),
        "maskbc": np.ascontiguousarray(maskbc.astype(ml_dtypes.bfloat16)),
        "ident": np.ascontiguousarray(np.eye(128, dtype=np.float32).astype(ml_dtypes.bfloat16)),
    }
    return w, beta


CHUNK = 512

_PROG_CACHE = {}


def kernel(**inputs):
    """Full-input entry point: shard batch over the 8 NeuronCores (one batch
    element per core; the scan state is per-(batch,channel) so this is
    embarrassingly parallel), run the Bass program SPMD, regather."""
    w, beta = prep_weights(inputs)
    x = np.asarray(inputs["x"], np.float32)
    v = np.asarray(inputs["velocity"], np.float32)
    n_cores, L, _ = x.shape
    key = (L, CHUNK, beta)
    if key not in _PROG_CACHE:
        _PROG_CACHE[key] = build_program(L, CHUNK, beta)
    nc = _PROG_CACHE[key]
    in_maps = []
    for b in range(n_cores):
        m = dict(w)
        m["x"] = np.ascontiguousarray(x[b])
        m["v"] = np.ascontiguousarray(v[b])
        in_maps.append(m)
    res = run_bass_kernel_spmd(nc, in_maps, core_ids=list(range(n_cores)))
    x_out = np.stack([res.results[b]["x_out"] for b in range(n_cores)])
    v_out = np.stack([res.results[b]["v_out"] for b in range(n_cores)])
    return (x_out, v_out)
